# revision 4
# baseline (speedup 1.0000x reference)
"""Per-core causal self-attention kernel (Bass/Tile, TRN2), v2.

One core's shard (batch b, head-group of HL=8 heads, reference quirk q=k=v):
    K  = x @ (32*Wk) + 32*bk              # [T, NW], NW = HL*64, scaled by 32
    per head h: S = K_h K_h^T / (8*1024) (causal), P = softmax rows
    Y_h = P @ K_h                          # carries the x32 scale
    out_partial = Y @ (Wp/32)              # [T, COUT]; host sums partials

Speed structure vs v1:
  - K-gen in fp8 e4m3 hi/lo from host (x = x8hi+x8lo, W = w8hi+w8lo, W
    pre-scaled x32 so the lo parts stay out of fp8 subnormals), computed as
    3 DoubleRow matmuls per 256-deep chunk pair: hi@hi + lo@hi + hi@lo.
    25% fewer PE cycles than fp16.
  - S strips via ONE DoubleRow fp8 matmul per head: lhsT k-tiles are
    (kt8_hi, kt8_lo) of the j-block (exact to ~0.4%), rhs is kt8_hi of the
    i-columns broadcast over the k-tile dim (stride-0).  0.5 cycles/col,
    2x faster than fp16; one-sided fp8 error ~1e-2 total (gate 2e-2).
  - PV + denominators unchanged fp16: lhsT = V slot [K_e|ones] / [ones|K_o],
    rhs = E^T.  V slot layout [ones|K_e|K_o|ones] lets ONE [128,128] DVE
    copy per (pair, tb) build it from the PE transpose of kt.
  - tri-mask mults, fp16->fp8 converts on GpSimd (Pool) to unload DVE.
  - Emission is ci-major (softmax column-chunk major) with JIT kt chunks
    and V pieces, and a 2-strip software pipeline (S of strip i+2 emitted
    before PV of strip i) so the in-order PE never waits on ACT exp.
    proj of chunk ci's 4 row-blocks happens right after chunk ci, spreading
    proj PE work instead of tail-loading it.

Engine budget (cycles per core): PE 332k @2.4GHz = 138us, ACT (exp)
146us <- wall, DVE ~103us, Pool ~95us, DMA ~41us.
"""

from contextlib import ExitStack

import concourse.bass as bass
import concourse.tile as tile
from concourse import mybir

F32 = mybir.dt.float32
BF16 = mybir.dt.bfloat16
FP16 = mybir.dt.float16
FP8 = mybir.dt.float8e4
EXP = mybir.ActivationFunctionType.Exp
DR = mybir.MatmulPerfMode.DoubleRow

WS = 32.0  # host-side scale on Wk/bk (keeps fp8 lo-parts normal); /WS on Wp


class Cfg:
    def __init__(self, T=2048, CIN=1024, HL=8, COUT=1024):
        self.T, self.CIN, self.HL, self.COUT = T, CIN, HL, COUT
        assert HL % 2 == 0 and T % 512 == 0 and CIN % 256 == 0 and COUT % 512 == 0
        self.NW = HL * 64          # local head dims
        self.NB = self.NW // 128   # head-pair blocks (4)
        self.TB = T // 128         # t row-blocks (16)
        self.NCH = T // 512        # i chunks (4)
        self.CP = CIN // 256       # contraction chunk-pairs for DR K-gen (4)


def declare_io(nc, cfg):
    io = {}
    io["x8hi"] = nc.dram_tensor("x8hi", [cfg.CIN, cfg.T], FP8, kind="ExternalInput")
    io["x8lo"] = nc.dram_tensor("x8lo", [cfg.CIN, cfg.T], FP8, kind="ExternalInput")
    io["w8hi"] = nc.dram_tensor("w8hi", [cfg.CIN, cfg.NW], FP8, kind="ExternalInput")
    io["w8lo"] = nc.dram_tensor("w8lo", [cfg.CIN, cfg.NW], FP8, kind="ExternalInput")
    io["bk"] = nc.dram_tensor("bk", [cfg.NW, 1], F32, kind="ExternalInput")
    io["wp"] = nc.dram_tensor("wp", [cfg.NW, cfg.COUT], FP16, kind="ExternalInput")
    io["tri"] = nc.dram_tensor("tri", [128, 128], FP16, kind="ExternalInput")
    io["ident"] = nc.dram_tensor("ident", [128, 128], FP16, kind="ExternalInput")
    io["out"] = nc.dram_tensor("out", [cfg.T, cfg.COUT], F32, kind="ExternalOutput")
    return io


def build(ctx: ExitStack, tc: tile.TileContext, io, cfg: Cfg):
    nc = tc.nc
    T, HL, NB, TB, NCH, CP, COUT = (cfg.T, cfg.HL, cfg.NB, cfg.TB, cfg.NCH,
                                    cfg.CP, cfg.COUT)

    consts = ctx.enter_context(tc.tile_pool(name="consts", bufs=1))
    # PSUM (8 banks): s 2x[128,1024]f32=4, u 2x[128,512]f32=2, k 2x[128,512]=2
    spsum = ctx.enter_context(tc.tile_pool(name="sps", bufs=2, space="PSUM"))
    upsum = ctx.enter_context(tc.tile_pool(name="ups", bufs=2, space="PSUM"))
    kpsum = ctx.enter_context(tc.tile_pool(name="kps", bufs=2, space="PSUM"))
    upool = ctx.enter_context(tc.tile_pool(name="usb", bufs=4))
    epool = ctx.enter_context(tc.tile_pool(name="e", bufs=8))
    rpool = ctx.enter_context(tc.tile_pool(name="r", bufs=4))
    opool = ctx.enter_context(tc.tile_pool(name="o", bufs=3))

    # ---- persistent SBUF tensors ----
    tri_t = consts.tile([128, 128], FP16, tag="tri")
    nc.sync.dma_start(tri_t[:], io["tri"].ap())
    id_t = consts.tile([128, 128], FP16, tag="ident")
    nc.sync.dma_start(id_t[:], io["ident"].ap())

    xp, wk8, bk_t, wp_t = {}, {}, [], []
    for hl in ("hi", "lo"):
        for cp in range(CP):
            t = consts.tile([128, 2, T], FP8, tag=f"x{hl}{cp}", name=f"x{hl}{cp}")
            for k in (0, 1):
                c0 = (2 * cp + k) * 128
                nc.sync.dma_start(t[:, k, :], io[f"x8{hl}"].ap()[c0:c0 + 128, :])
            xp[hl, cp] = t
            t = consts.tile([128, 2, cfg.NW], FP8, tag=f"w{hl}{cp}", name=f"w{hl}{cp}")
            for k in (0, 1):
                c0 = (2 * cp + k) * 128
                nc.sync.dma_start(t[:, k, :], io[f"w8{hl}"].ap()[c0:c0 + 128, :])
            wk8[hl, cp] = t
    kt_t, kt8_t, y_t, v_t = [], [], [], []
    for nb in range(NB):
        t = consts.tile([128, 1], F32, tag=f"bk{nb}", name=f"bk{nb}")
        nc.sync.dma_start(t[:], io["bk"].ap()[nb * 128:(nb + 1) * 128, :])
        bk_t.append(t)
        t = consts.tile([128, COUT], FP16, tag=f"wp{nb}", name=f"wp{nb}")
        nc.sync.dma_start(t[:], io["wp"].ap()[nb * 128:(nb + 1) * 128, :])
        wp_t.append(t)
        kt_t.append(consts.tile([128, T], FP16, tag=f"kt{nb}", name=f"kt{nb}"))
        kt8_t.append(consts.tile([128, 2, T], FP8, tag=f"k8{nb}", name=f"k8{nb}"))
        y_t.append(consts.tile([128, T], FP16, tag=f"y{nb}", name=f"y{nb}"))
        # V slots per t-block: [ones(64) | K_e(64) | K_o(64) | ones(64)];
        # even-head lhsT = cols 0:128 -> rows [l_e; U_e],
        # odd-head  lhsT = cols 128:256 -> rows [U_o; l_o].
        v_t.append([consts.tile([128, 256], FP16, tag=f"v{nb}_{tb}",
                                name=f"v{nb}_{tb}") for tb in range(TB)])

    # warm the ACT exp table off the critical path
    warm = consts.tile([128, 1], F32, tag="warm", name="warm")
    nc.gpsimd.memset(warm[:], 0.0)
    nc.scalar.activation(warm[:], warm[:], EXP, scale=1.0)

    def kt_chunk(nb, tch):
        # KT[n, 512-chunk] = (x @ Wk*WS + bk*WS)^T via 12 DoubleRow matmuls,
        # then bias+cast (DVE) and fp8 hi/lo split (Pool).
        cols = slice(tch * 512, (tch + 1) * 512)
        ps = kpsum.tile([128, 512], F32, tag="kps", name="pskt")
        nsl = slice(nb * 128, (nb + 1) * 128)
        for cp in range(CP):
            for i, (wv, xv) in enumerate((("hi", "hi"), ("lo", "hi"), ("hi", "lo"))):
                nc.tensor.matmul(
                    ps[:], wk8[wv, cp][:, :, nsl], xp[xv, cp][:, :, cols],
                    start=(cp == 0 and i == 0), stop=(cp == CP - 1 and i == 2),
                    perf_mode=DR)
        nc.vector.tensor_scalar_add(kt_t[nb][:, cols], ps[:], bk_t[nb][:])
        nc.gpsimd.tensor_copy(kt8_t[nb][:, 0, cols], kt_t[nb][:, cols])
        nc.gpsimd.tensor_sub(kt8_t[nb][:, 1, cols], kt_t[nb][:, cols],
                             kt8_t[nb][:, 0, cols])

    def v_piece(nb, tb):
        # ones borders (Pool) + K middle from PE transpose, one DVE copy
        nc.gpsimd.memset(v_t[nb][tb][:, 0:64], 1.0)
        nc.gpsimd.memset(v_t[nb][tb][:, 192:256], 1.0)
        ps = kpsum.tile([128, 128], FP16, tag="kps", name="pst")
        nc.tensor.transpose(ps[:], kt_t[nb][:, tb * 128:(tb + 1) * 128], id_t[:])
        nc.vector.tensor_copy(v_t[nb][tb][:, 64:192], ps[:])

    def strip_S(hp, ci, jb):
        # S^T strip [j-block jb, i-cols of chunk ci] for both heads of pair hp
        off = max(0, 128 * jb - 512 * ci)
        N = 512 - off
        ilo = 512 * ci + off
        ps = spsum.tile([128, 1024], F32, tag="sps", name="psS")
        for h, rows in ((0, slice(0, 64)), (1, slice(64, 128))):
            lhsT = kt8_t[hp][rows, :, jb * 128:(jb + 1) * 128]
            rhs = (kt8_t[hp][rows, 0, ilo:ilo + N]
                   .unsqueeze(1).broadcast_to([64, 2, N]))
            nc.tensor.matmul(ps[:, h * 512:h * 512 + N], lhsT, rhs,
                             start=True, stop=True, perf_mode=DR)
        et = epool.tile([128, 1024], FP16, tag="e", name="et")
        nc.scalar.activation(
            et[:].rearrange("p (a c) -> p a c", a=2)[:, :, 0:N],
            ps[:].rearrange("p (a c) -> p a c", a=2)[:, :, 0:N],
            EXP, scale=0.125 / (WS * WS))
        if jb >= 4 * ci:  # strip starts at the causal diagonal block
            nc.gpsimd.tensor_mul(et[:, 0:128], et[:, 0:128], tri_t[:])
            nc.gpsimd.tensor_mul(et[:, 512:640], et[:, 512:640], tri_t[:])
        return et, off, N

    def strip_PV(hp, ci, jb, et, off, N, uA, uB, jmax):
        st, sp = (jb == 0), (jb == jmax - 1)
        nc.tensor.matmul(uA[:, off:off + N], v_t[hp][jb][:, 0:128],
                         et[:, 0:N], start=st, stop=sp)
        nc.tensor.matmul(uB[:, off:off + N], v_t[hp][jb][:, 128:256],
                         et[:, 512:512 + N], start=st, stop=sp)

    def normalize(hp, ci, uA, uB):
        # uA rows: [l_e; U_e], uB rows: [U_o; l_o]; y rows: [U_o/l_o; U_e/l_e]
        cs = slice(ci * 512, (ci + 1) * 512)
        usA = upool.tile([128, 512], F32, tag="us", name="usA")
        nc.vector.tensor_copy(usA[:], uA[:])
        usB = upool.tile([128, 512], F32, tag="us", name="usB")
        nc.vector.tensor_copy(usB[:], uB[:])
        rA = rpool.tile([128, 512], F32, tag="r", name="rA")
        nc.vector.reciprocal(rA[0:64, :], usA[0:64, :])
        nc.sync.dma_start(rA[64:128, :], rA[0:64, :])
        nc.vector.tensor_mul(y_t[hp][64:128, cs], usA[64:128, :], rA[64:128, :])
        rB = rpool.tile([128, 512], F32, tag="r", name="rB")
        nc.vector.reciprocal(rB[64:128, :], usB[64:128, :])
        nc.sync.dma_start(rB[0:64, :], rB[64:128, :])
        nc.vector.tensor_mul(y_t[hp][0:64, cs], usB[0:64, :], rB[0:64, :])

    def proj_tb(tb):
        ot = opool.tile([128, COUT], F32, tag="o", name="ot")
        for nh in range(COUT // 512):
            po = kpsum.tile([128, 512], F32, tag="kps", name="po")
            for hp2 in range(NB):
                nc.tensor.matmul(po[:], y_t[hp2][:, tb * 128:(tb + 1) * 128],
                                 wp_t[hp2][:, nh * 512:(nh + 1) * 512],
                                 start=(hp2 == 0), stop=(hp2 == NB - 1))
            nc.vector.tensor_copy(ot[:, nh * 512:(nh + 1) * 512], po[:])
        nc.sync.dma_start(io["out"].ap()[tb * 128:(tb + 1) * 128, :], ot[:])

    LA = 2  # strip software-pipeline lookahead
    for ci in range(NCH):
        for hp in range(NB):
            kt_chunk(hp, ci)
            for tb in range(4 * ci, 4 * ci + 4):
                v_piece(hp, tb)
            jmax = (ci + 1) * 4
            uA = upsum.tile([128, 512], F32, tag="u", name="uA")
            uB = upsum.tile([128, 512], F32, tag="u", name="uB")
            pend = []
            for jb in range(jmax):
                pend.append((jb,) + strip_S(hp, ci, jb))
                if len(pend) > LA:
                    jb0, et, off, N = pend.pop(0)
                    strip_PV(hp, ci, jb0, et, off, N, uA, uB, jmax)
            for jb0, et, off, N in pend:
                strip_PV(hp, ci, jb0, et, off, N, uA, uB, jmax)
            normalize(hp, ci, uA, uB)
        for tb in range(4 * ci, 4 * ci + 4):
            proj_tb(tb)


def make_inputs(cfg, x, Wk, bk, Wp):
    """Host-side input map for one core.
    x [T,CIN] fp32, Wk [CIN,NW], bk [NW], Wp [NW,COUT] (natural head order)."""
    import numpy as np
    import ml_dtypes
    E4 = ml_dtypes.float8_e4m3fn
    xT = np.ascontiguousarray(x.T).astype(np.float32)
    x8hi = xT.astype(E4)
    x8lo = (xT - x8hi.astype(np.float32)).astype(E4)
    Wks = (Wk * WS).astype(np.float32)
    w8hi = Wks.astype(E4)
    w8lo = (Wks - w8hi.astype(np.float32)).astype(E4)
    # wp rows per pair: [odd-head dims; even-head dims] to match y layout
    wp = (Wp / WS).astype(np.float32).reshape(cfg.NB, 2, 64, cfg.COUT)
    wp = np.ascontiguousarray(wp[:, ::-1].reshape(cfg.NW, cfg.COUT))
    jj, ii = np.meshgrid(np.arange(128), np.arange(128), indexing="ij")
    return {
        "x8hi": x8hi, "x8lo": x8lo, "w8hi": w8hi, "w8lo": w8lo,
        "bk": (bk * WS).reshape(-1, 1).astype(np.float32),
        "wp": wp.astype(np.float16),
        "tri": (jj <= ii).astype(np.float16),
        "ident": np.eye(128).astype(np.float16),
    }


# ======================================================================
# Host-side entry: shard across 8 NeuronCores as (batch x head-group),
# run the Bass kernel, gather + reduce partials on host.
# ======================================================================

import numpy as np

from concourse import bacc
from concourse.bass_utils import run_bass_kernel_spmd

B, T, C, H = 4, 2048, 1024, 16
N_CORES = 8
HG = 2                      # head groups (tensor-parallel axis)
NW = C // HG                # 512 columns of W_k per group

_cache = {}


def get_compiled():
    if "nc" not in _cache:
        cfg = Cfg(T=T, CIN=C, HL=H // HG, COUT=C)
        nc = bacc.Bacc("TRN2", target_bir_lowering=False, debug=False,
                       num_devices=N_CORES)
        io = declare_io(nc, cfg)
        with tile.TileContext(nc) as tc:
            with ExitStack() as ctx:
                build(ctx, tc, io, cfg)
        nc.compile()
        _cache["nc"] = (nc, cfg)
    return _cache["nc"]


def make_in_maps(cfg, x, W_attn, b_attn, W_proj):
    in_maps = []
    for core in range(N_CORES):
        b, hg = core // HG, core % HG
        sl = slice(C + hg * NW, C + (hg + 1) * NW)
        in_maps.append(make_inputs(
            cfg, x[b], W_attn[:, sl], b_attn[sl],
            W_proj[hg * NW:(hg + 1) * NW, :]))
    return in_maps


def kernel(x, W_attn, b_attn, W_proj, b_proj):
    x = np.asarray(x, dtype=np.float32)
    W_attn = np.asarray(W_attn, dtype=np.float32)
    b_attn = np.asarray(b_attn, dtype=np.float32)
    W_proj = np.asarray(W_proj, dtype=np.float32)
    b_proj = np.asarray(b_proj, dtype=np.float32)

    nc, cfg = get_compiled()
    in_maps = make_in_maps(cfg, x, W_attn, b_attn, W_proj)
    res = run_bass_kernel_spmd(nc, in_maps, core_ids=list(range(N_CORES)))
    out = np.empty((B, T, C), dtype=np.float32)
    for b in range(B):
        out[b] = res.results[HG * b]["out"] + res.results[HG * b + 1]["out"] \
            + b_proj[None, :]
    return out


# revision 9
# speedup vs baseline: 1.1766x; 1.1766x over previous
"""Per-core causal self-attention kernel (Bass/Tile, TRN2), v2.

One core's shard (batch b, head-group of HL=8 heads, reference quirk q=k=v):
    K  = x @ (32*Wk) + 32*bk              # [T, NW], NW = HL*64, scaled by 32
    per head h: S = K_h K_h^T / (8*1024) (causal), P = softmax rows
    Y_h = P @ K_h                          # carries the x32 scale
    out_partial = Y @ (Wp/32)              # [T, COUT]; host sums partials

Speed structure vs v1:
  - K-gen in fp8 e4m3 hi/lo from host (x = x8hi+x8lo, W = w8hi+w8lo, W
    pre-scaled x32 so the lo parts stay out of fp8 subnormals), computed as
    3 DoubleRow matmuls per 256-deep chunk pair: hi@hi + lo@hi + hi@lo.
    25% fewer PE cycles than fp16.
  - S strips via ONE DoubleRow fp8 matmul per head: lhsT k-tiles are
    (kt8_hi, kt8_lo) of the j-block (exact to ~0.4%), rhs is kt8_hi of the
    i-columns broadcast over the k-tile dim (stride-0).  0.5 cycles/col,
    2x faster than fp16; one-sided fp8 error ~1e-2 total (gate 2e-2).
  - PV + denominators unchanged fp16: lhsT = V slot [K_e|ones] / [ones|K_o],
    rhs = E^T.  V slot layout [ones|K_e|K_o|ones] lets ONE [128,128] DVE
    copy per (pair, tb) build it from the PE transpose of kt.
  - tri-mask mults, fp16->fp8 converts on GpSimd (Pool) to unload DVE.
  - Emission is ci-major (softmax column-chunk major) with JIT kt chunks
    and V pieces, and a 2-strip software pipeline (S of strip i+2 emitted
    before PV of strip i) so the in-order PE never waits on ACT exp.
    proj of chunk ci's 4 row-blocks happens right after chunk ci, spreading
    proj PE work instead of tail-loading it.

Engine budget (cycles per core): PE 332k @2.4GHz = 138us, ACT (exp)
146us <- wall, DVE ~103us, Pool ~95us, DMA ~41us.
"""

from contextlib import ExitStack

import concourse.bass as bass
import concourse.tile as tile
from concourse import mybir

F32 = mybir.dt.float32
BF16 = mybir.dt.bfloat16
FP16 = mybir.dt.float16
FP8 = mybir.dt.float8e4
EXP = mybir.ActivationFunctionType.Exp
DR = mybir.MatmulPerfMode.DoubleRow

WS = 32.0  # host-side scale on Wk/bk (keeps fp8 lo-parts normal); /WS on Wp


class Cfg:
    def __init__(self, T=2048, CIN=1024, HL=8, COUT=1024):
        self.T, self.CIN, self.HL, self.COUT = T, CIN, HL, COUT
        assert HL % 2 == 0 and T % 512 == 0 and CIN % 256 == 0 and COUT % 512 == 0
        self.NW = HL * 64          # local head dims
        self.NB = self.NW // 128   # head-pair blocks (4)
        self.TB = T // 128         # t row-blocks (16)
        self.NCH = T // 512        # i chunks (4)
        self.CP = CIN // 256       # contraction chunk-pairs for DR K-gen (4)


def declare_io(nc, cfg):
    io = {}
    io["x8hi"] = nc.dram_tensor("x8hi", [cfg.CIN, cfg.T], FP8, kind="ExternalInput")
    io["x8lo"] = nc.dram_tensor("x8lo", [cfg.CIN, cfg.T], FP8, kind="ExternalInput")
    io["w8hi"] = nc.dram_tensor("w8hi", [cfg.CIN, cfg.NW], FP8, kind="ExternalInput")
    io["w8lo"] = nc.dram_tensor("w8lo", [cfg.CIN, cfg.NW], FP8, kind="ExternalInput")
    io["bk"] = nc.dram_tensor("bk", [cfg.NW, 1], F32, kind="ExternalInput")
    io["wp"] = nc.dram_tensor("wp", [cfg.NW, cfg.COUT], FP16, kind="ExternalInput")
    io["tri"] = nc.dram_tensor("tri", [128, 128], FP16, kind="ExternalInput")
    io["ident"] = nc.dram_tensor("ident", [128, 128], FP16, kind="ExternalInput")
    io["out"] = nc.dram_tensor("out", [cfg.T, cfg.COUT], F32, kind="ExternalOutput")
    return io


def build(ctx: ExitStack, tc: tile.TileContext, io, cfg: Cfg):
    nc = tc.nc
    T, HL, NB, TB, NCH, CP, COUT = (cfg.T, cfg.HL, cfg.NB, cfg.TB, cfg.NCH,
                                    cfg.CP, cfg.COUT)

    consts = ctx.enter_context(tc.tile_pool(name="consts", bufs=1))
    # PSUM (8 banks): s 2x[128,1024]f32=4, u 2x[128,512]f32=2, k 2x[128,512]=2
    spsum = ctx.enter_context(tc.tile_pool(name="sps", bufs=2, space="PSUM"))
    upsum = ctx.enter_context(tc.tile_pool(name="ups", bufs=2, space="PSUM"))
    kpsum = ctx.enter_context(tc.tile_pool(name="kps", bufs=2, space="PSUM"))
    upool = ctx.enter_context(tc.tile_pool(name="usb", bufs=4))
    epool = ctx.enter_context(tc.tile_pool(name="e", bufs=8))
    rpool = ctx.enter_context(tc.tile_pool(name="r", bufs=4))
    opool = ctx.enter_context(tc.tile_pool(name="o", bufs=3))

    # ---- persistent SBUF tensors ----
    tri_t = consts.tile([128, 128], FP16, tag="tri")
    nc.sync.dma_start(tri_t[:], io["tri"].ap())
    id_t = consts.tile([128, 128], FP16, tag="ident")
    nc.sync.dma_start(id_t[:], io["ident"].ap())

    xp, wk8, bk_t, wp_t = {}, {}, [], []
    for hl in ("hi", "lo"):
        for cp in range(CP):
            xp[hl, cp] = consts.tile([128, 2, T], FP8, tag=f"x{hl}{cp}",
                                     name=f"x{hl}{cp}")
            t = consts.tile([128, 2, cfg.NW], FP8, tag=f"w{hl}{cp}", name=f"w{hl}{cp}")
            for k in (0, 1):
                c0 = (2 * cp + k) * 128
                nc.sync.dma_start(t[:, k, :], io[f"w8{hl}"].ap()[c0:c0 + 128, :])
            wk8[hl, cp] = t

    def x_dma_chunk(tch):
        # JIT column chunk of the fp8 x inputs (used by kt chunks of ci=tch)
        cols = slice(tch * 512, (tch + 1) * 512)
        for hl in ("hi", "lo"):
            for cp in range(CP):
                for k in (0, 1):
                    c0 = (2 * cp + k) * 128
                    nc.sync.dma_start(xp[hl, cp][:, k, cols],
                                      io[f"x8{hl}"].ap()[c0:c0 + 128, cols])
    kt_t, kt8_t, y_t, v_t = [], [], [], []
    for nb in range(NB):
        t = consts.tile([128, 1], F32, tag=f"bk{nb}", name=f"bk{nb}")
        nc.sync.dma_start(t[:], io["bk"].ap()[nb * 128:(nb + 1) * 128, :])
        bk_t.append(t)
        wp_t.append(consts.tile([128, COUT], FP16, tag=f"wp{nb}", name=f"wp{nb}"))
        kt_t.append(consts.tile([128, T], FP16, tag=f"kt{nb}", name=f"kt{nb}"))
        kt8_t.append(consts.tile([128, 2, T], FP8, tag=f"k8{nb}", name=f"k8{nb}"))
        y_t.append(consts.tile([128, T], FP16, tag=f"y{nb}", name=f"y{nb}"))
        # V slots per t-block: [ones(64) | K_e(64) | K_o(64) | ones(64)];
        # even-head lhsT = cols 0:128 -> rows [l_e; U_e],
        # odd-head  lhsT = cols 128:256 -> rows [U_o; l_o].
        v_t.append([consts.tile([128, 256], FP16, tag=f"v{nb}_{tb}",
                                name=f"v{nb}_{tb}") for tb in range(TB)])

    x_dma_chunk(0)

    def wp_dma():
        # wp is only needed by proj (~40us in); keep it off the startup queue
        for nb in range(NB):
            nc.sync.dma_start(wp_t[nb][:],
                              io["wp"].ap()[nb * 128:(nb + 1) * 128, :])

    # warm the ACT exp table off the critical path
    warm = consts.tile([128, 1], F32, tag="warm", name="warm")
    nc.gpsimd.memset(warm[:], 0.0)
    nc.scalar.activation(warm[:], warm[:], EXP, scale=1.0)

    def kt_chunk(nb, tch):
        # KT[n, 512-chunk] = (x @ Wk*WS + bk*WS)^T via 12 DoubleRow matmuls,
        # then bias+cast (DVE) and fp8 hi/lo split (Pool).
        cols = slice(tch * 512, (tch + 1) * 512)
        ps = kpsum.tile([128, 512], F32, tag="kps", name="pskt")
        nsl = slice(nb * 128, (nb + 1) * 128)
        for cp in range(CP):
            for i, (wv, xv) in enumerate((("hi", "hi"), ("lo", "hi"), ("hi", "lo"))):
                nc.tensor.matmul(
                    ps[:], wk8[wv, cp][:, :, nsl], xp[xv, cp][:, :, cols],
                    start=(cp == 0 and i == 0), stop=(cp == CP - 1 and i == 2),
                    perf_mode=DR)
        nc.vector.tensor_scalar_add(kt_t[nb][:, cols], ps[:], bk_t[nb][:])
        nc.gpsimd.tensor_copy(kt8_t[nb][:, 0, cols], kt_t[nb][:, cols])
        nc.gpsimd.tensor_sub(kt8_t[nb][:, 1, cols], kt_t[nb][:, cols],
                             kt8_t[nb][:, 0, cols])

    def v_piece(nb, tb):
        # ones borders (Pool) + K middle from PE transpose, one DVE copy
        nc.gpsimd.memset(v_t[nb][tb][:, 0:64], 1.0)
        nc.gpsimd.memset(v_t[nb][tb][:, 192:256], 1.0)
        ps = kpsum.tile([128, 128], FP16, tag="kps", name="pst")
        nc.tensor.transpose(ps[:], kt_t[nb][:, tb * 128:(tb + 1) * 128], id_t[:])
        nc.vector.tensor_copy(v_t[nb][tb][:, 64:192], ps[:])

    SHIFT = list(range(32))  # identity mask: copies partition window as-is

    def strip_S(hp, ci, jb):
        # S^T strip [j-block jb, i-cols of chunk ci] for both heads of pair hp
        off = max(0, 128 * jb - 512 * ci)
        N = 512 - off
        ilo = 512 * ci + off
        ps = spsum.tile([128, 1024], F32, tag="sps", name="psS")
        for h, rows in ((0, slice(0, 64)), (1, slice(64, 128))):
            lhsT = kt8_t[hp][rows, :, jb * 128:(jb + 1) * 128]
            rhs = (kt8_t[hp][rows, 0, ilo:ilo + N]
                   .unsqueeze(1).broadcast_to([64, 2, N]))
            nc.tensor.matmul(ps[:, h * 512:h * 512 + N], lhsT, rhs,
                             start=True, stop=True, perf_mode=DR)
        et = epool.tile([128, 1024], FP16, tag="e", name="et")
        nc.scalar.activation(
            et[:].rearrange("p (a c) -> p a c", a=2)[:, :, 0:N],
            ps[:].rearrange("p (a c) -> p a c", a=2)[:, :, 0:N],
            EXP, scale=0.125 / (WS * WS))
        if jb >= 4 * ci:  # strip starts at the causal diagonal block
            nc.gpsimd.tensor_mul(et[:, 0:128], et[:, 0:128], tri_t[:])
            nc.gpsimd.tensor_mul(et[:, 512:640], et[:, 512:640], tri_t[:])
        return et, off, N

    def strip_PV(hp, ci, jb, et, off, N, uA, uB, jmax):
        st, sp = (jb == 0), (jb == jmax - 1)
        nc.tensor.matmul(uA[:, off:off + N], v_t[hp][jb][:, 0:128],
                         et[:, 0:N], start=st, stop=sp)
        nc.tensor.matmul(uB[:, off:off + N], v_t[hp][jb][:, 128:256],
                         et[:, 512:512 + N], start=st, stop=sp)

    def normalize(hp, ci, uA, uB, drains):
        # uA rows: [l_e; U_e], uB rows: [U_o; l_o]; y rows: [U_o/l_o; U_e/l_e]
        # us-copies first (frees the u PSUM bank pair for the next unit),
        # then the pending proj drain, then recip/shift/scale on SBUF.
        cs = slice(ci * 512, (ci + 1) * 512)
        usA = upool.tile([128, 512], F32, tag="us", name="usA")
        nc.vector.tensor_copy(usA[:], uA[:])
        usB = upool.tile([128, 512], F32, tag="us", name="usB")
        nc.vector.tensor_copy(usB[:], uB[:])
        for d in drains:
            d()
        rA = rpool.tile([128, 512], F32, tag="r", name="rA")
        nc.vector.reciprocal(rA[0:64, :], usA[0:64, :])
        rB = rpool.tile([128, 512], F32, tag="r", name="rB")
        nc.vector.reciprocal(rB[64:128, :], usB[64:128, :])
        nc.vector.stream_shuffle(rA[64:128, :], rA[0:64, :], SHIFT)
        nc.vector.stream_shuffle(rB[0:64, :], rB[64:128, :], SHIFT)
        nc.vector.tensor_mul(y_t[hp][64:128, cs], usA[64:128, :], rA[64:128, :])
        nc.vector.tensor_mul(y_t[hp][0:64, cs], usB[0:64, :], rB[0:64, :])

    def proj_mm(tb):
        # PE part of one proj row-block; returns a drain closure (DVE copies
        # + out DMA) that normalize() runs after its PSUM-freeing us-copies.
        pos = []
        for nh in range(COUT // 512):
            po = kpsum.tile([128, 512], F32, tag="kps", name="po")
            for hp2 in range(NB):
                nc.tensor.matmul(po[:], y_t[hp2][:, tb * 128:(tb + 1) * 128],
                                 wp_t[hp2][:, nh * 512:(nh + 1) * 512],
                                 start=(hp2 == 0), stop=(hp2 == NB - 1))
            pos.append(po)

        def drain():
            ot = opool.tile([128, COUT], F32, tag="o", name="ot")
            for nh, po in enumerate(pos):
                nc.vector.tensor_copy(ot[:, nh * 512:(nh + 1) * 512], po[:])
            nc.sync.dma_start(io["out"].ap()[tb * 128:(tb + 1) * 128, :], ot[:])
        return drain

    LA = 2  # strip software-pipeline lookahead
    units = [(ci, hp) for ci in range(NCH) for hp in range(NB)]
    kt_chunk(units[0][1], units[0][0])
    for tb in range(4):
        v_piece(units[0][1], tb)
    proj_pend = []   # tbs whose proj matmuls still need emitting
    for k, (ci, hp) in enumerate(units):
        jmax = (ci + 1) * 4
        uA = upsum.tile([128, 512], F32, tag="u", name="uA")
        uB = upsum.tile([128, 512], F32, tag="u", name="uB")
        pend = []
        for idx, jb in enumerate(range(jmax)):
            pend.append((jb,) + strip_S(hp, ci, jb))
            if idx == 1 and k + 1 < len(units):
                # one-unit-lookahead prep: next unit's kt chunk (and x DMA /
                # wp DMA staging at ci boundaries)
                nci, nhp = units[k + 1]
                if nhp == 0:
                    x_dma_chunk(nci)
                if k == 1:
                    wp_dma()
                kt_chunk(nhp, nci)
            if idx == 3 and k + 1 < len(units):
                nci, nhp = units[k + 1]
                for tb in range(4 * nci, 4 * nci + 4):
                    v_piece(nhp, tb)
            if len(pend) > LA:
                jb0, et, off, N = pend.pop(0)
                strip_PV(hp, ci, jb0, et, off, N, uA, uB, jmax)
        for jb0, et, off, N in pend:
            strip_PV(hp, ci, jb0, et, off, N, uA, uB, jmax)
        drains = []
        if proj_pend:
            drains.append(proj_mm(proj_pend.pop(0)))
        normalize(hp, ci, uA, uB, drains)
        if hp == NB - 1:
            proj_pend += list(range(4 * ci, 4 * ci + 4))
    # tail: remaining proj row-blocks of the last chunk
    for tb in proj_pend:
        d = proj_mm(tb)
        d()


def make_inputs(cfg, x, Wk, bk, Wp):
    """Host-side input map for one core.
    x [T,CIN] fp32, Wk [CIN,NW], bk [NW], Wp [NW,COUT] (natural head order)."""
    import numpy as np
    import ml_dtypes
    E4 = ml_dtypes.float8_e4m3fn
    xT = np.ascontiguousarray(x.T).astype(np.float32)
    x8hi = xT.astype(E4)
    x8lo = (xT - x8hi.astype(np.float32)).astype(E4)
    Wks = (Wk * WS).astype(np.float32)
    w8hi = Wks.astype(E4)
    w8lo = (Wks - w8hi.astype(np.float32)).astype(E4)
    # wp rows per pair: [odd-head dims; even-head dims] to match y layout
    wp = (Wp / WS).astype(np.float32).reshape(cfg.NB, 2, 64, cfg.COUT)
    wp = np.ascontiguousarray(wp[:, ::-1].reshape(cfg.NW, cfg.COUT))
    jj, ii = np.meshgrid(np.arange(128), np.arange(128), indexing="ij")
    return {
        "x8hi": x8hi, "x8lo": x8lo, "w8hi": w8hi, "w8lo": w8lo,
        "bk": (bk * WS).reshape(-1, 1).astype(np.float32),
        "wp": wp.astype(np.float16),
        "tri": (jj <= ii).astype(np.float16),
        "ident": np.eye(128).astype(np.float16),
    }


# ======================================================================
# Host-side entry: shard across 8 NeuronCores as (batch x head-group),
# run the Bass kernel, gather + reduce partials on host.
# ======================================================================

import numpy as np

from concourse import bacc
from concourse.bass_utils import run_bass_kernel_spmd

B, T, C, H = 4, 2048, 1024, 16
N_CORES = 8
HG = 2                      # head groups (tensor-parallel axis)
NW = C // HG                # 512 columns of W_k per group

_cache = {}


def get_compiled():
    if "nc" not in _cache:
        cfg = Cfg(T=T, CIN=C, HL=H // HG, COUT=C)
        nc = bacc.Bacc("TRN2", target_bir_lowering=False, debug=False,
                       num_devices=N_CORES)
        io = declare_io(nc, cfg)
        with tile.TileContext(nc) as tc:
            with ExitStack() as ctx:
                build(ctx, tc, io, cfg)
        nc.compile()
        _cache["nc"] = (nc, cfg)
    return _cache["nc"]


def make_in_maps(cfg, x, W_attn, b_attn, W_proj):
    in_maps = []
    for core in range(N_CORES):
        b, hg = core // HG, core % HG
        sl = slice(C + hg * NW, C + (hg + 1) * NW)
        in_maps.append(make_inputs(
            cfg, x[b], W_attn[:, sl], b_attn[sl],
            W_proj[hg * NW:(hg + 1) * NW, :]))
    return in_maps


def kernel(x, W_attn, b_attn, W_proj, b_proj):
    x = np.asarray(x, dtype=np.float32)
    W_attn = np.asarray(W_attn, dtype=np.float32)
    b_attn = np.asarray(b_attn, dtype=np.float32)
    W_proj = np.asarray(W_proj, dtype=np.float32)
    b_proj = np.asarray(b_proj, dtype=np.float32)

    nc, cfg = get_compiled()
    in_maps = make_in_maps(cfg, x, W_attn, b_attn, W_proj)
    res = run_bass_kernel_spmd(nc, in_maps, core_ids=list(range(N_CORES)))
    out = np.empty((B, T, C), dtype=np.float32)
    for b in range(B):
        out[b] = res.results[HG * b]["out"] + res.results[HG * b + 1]["out"] \
            + b_proj[None, :]
    return out


# revision 10
# speedup vs baseline: 1.2610x; 1.0717x over previous
"""Per-core causal self-attention kernel (Bass/Tile, TRN2), v4.

One core's shard (batch b, head-group of HL=8 heads, reference quirk q=k=v):
    K  = x @ (32*Wk) + 32*bk              # [T, NW], NW = HL*64, scaled x32
    per head h: S = K_h K_h^T / (8*1024) (causal), P = softmax rows
    Y_h = P @ K_h                          # carries the x32 scale
    out_partial = Y @ (Wp/32)              # [T, COUT]; host sums partials

Speed structure:
  - K-gen in fp8 e4m3 hi/lo from host (x = x8hi+x8lo, W = w8hi+w8lo; W is
    pre-scaled x32 so the lo residuals stay out of fp8 subnormals), computed
    as 3 DoubleRow matmuls per 256-deep chunk pair: hi@hi + lo@hi + hi@lo.
  - S strips via ONE DoubleRow fp8 matmul per head: lhsT k-tiles are
    (kt8_hi, kt8_lo) of the j-block (exact to ~0.4%), rhs is kt8_hi of the
    i-columns broadcast over the k-tile dim (stride 0).  0.5 cycles/col.
  - PV + denominators in fp16: lhsT = V slot [ones|K_e] / [K_o|ones] from
    layout [ones|K_e|K_o|ones], built by ONE [128,128] DVE copy per
    (pair, t-block) from the PE transpose of kt.
  - tri-mask mults and fp16->fp8 kt splits on GpSimd (Pool); softmax
    normalize uses DVE stream_shuffle for the partition shift (no DMA).
  - GLOBAL software pipeline over all (chunk ci, pair hp, j-block) strips:
    S of strip g+2 is emitted before PV of strip g, ACROSS unit boundaries,
    so the in-order PE never makes the ACT exp stream wait.  kt chunks, V
    pieces and x DMAs are emitted just-in-time one unit ahead; proj of
    chunk ci's row-blocks rides the units of chunk ci+1.

Engine budget per core: ACT (exp) ~147us <- wall, PE ~140us, DVE ~105us,
Pool ~95us, DMA ~45us.
"""

from contextlib import ExitStack

import concourse.bass as bass
import concourse.tile as tile
from concourse import mybir

F32 = mybir.dt.float32
FP16 = mybir.dt.float16
FP8 = mybir.dt.float8e4
EXP = mybir.ActivationFunctionType.Exp
DR = mybir.MatmulPerfMode.DoubleRow

WS = 32.0  # host-side scale on Wk/bk (keeps fp8 lo-parts normal); /WS on Wp


class Cfg:
    def __init__(self, T=2048, CIN=1024, HL=8, COUT=1024):
        self.T, self.CIN, self.HL, self.COUT = T, CIN, HL, COUT
        assert HL % 2 == 0 and T % 512 == 0 and CIN % 256 == 0 and COUT % 512 == 0
        self.NW = HL * 64          # local head dims
        self.NB = self.NW // 128   # head-pair blocks (4)
        self.TB = T // 128         # t row-blocks (16)
        self.NCH = T // 512        # i chunks (4)
        self.CP = CIN // 256       # contraction chunk-pairs for DR K-gen (4)


def declare_io(nc, cfg):
    io = {}
    io["x8hi"] = nc.dram_tensor("x8hi", [cfg.CIN, cfg.T], FP8, kind="ExternalInput")
    io["x8lo"] = nc.dram_tensor("x8lo", [cfg.CIN, cfg.T], FP8, kind="ExternalInput")
    io["w8hi"] = nc.dram_tensor("w8hi", [cfg.CIN, cfg.NW], FP8, kind="ExternalInput")
    io["w8lo"] = nc.dram_tensor("w8lo", [cfg.CIN, cfg.NW], FP8, kind="ExternalInput")
    io["bk"] = nc.dram_tensor("bk", [cfg.NW, 1], F32, kind="ExternalInput")
    io["wp"] = nc.dram_tensor("wp", [cfg.NW, cfg.COUT], FP16, kind="ExternalInput")
    io["tri"] = nc.dram_tensor("tri", [128, 128], FP16, kind="ExternalInput")
    io["ident"] = nc.dram_tensor("ident", [128, 128], FP16, kind="ExternalInput")
    io["out"] = nc.dram_tensor("out", [cfg.T, cfg.COUT], F32, kind="ExternalOutput")
    return io


def build(ctx: ExitStack, tc: tile.TileContext, io, cfg: Cfg):
    nc = tc.nc
    T, HL, NB, TB, NCH, CP, COUT = (cfg.T, cfg.HL, cfg.NB, cfg.TB, cfg.NCH,
                                    cfg.CP, cfg.COUT)

    consts = ctx.enter_context(tc.tile_pool(name="consts", bufs=1))
    # PSUM (8 banks): s 2x[128,1024]f32=4, u 2x[128,512]f32=2, k 2x[128,512]=2
    spsum = ctx.enter_context(tc.tile_pool(name="sps", bufs=2, space="PSUM"))
    upsum = ctx.enter_context(tc.tile_pool(name="ups", bufs=2, space="PSUM"))
    kpsum = ctx.enter_context(tc.tile_pool(name="kps", bufs=2, space="PSUM"))
    upool = ctx.enter_context(tc.tile_pool(name="usb", bufs=4))
    epool = ctx.enter_context(tc.tile_pool(name="e", bufs=8))
    rpool = ctx.enter_context(tc.tile_pool(name="r", bufs=4))
    opool = ctx.enter_context(tc.tile_pool(name="o", bufs=3))

    # ---- persistent SBUF tensors; DMA order favors the hi-path startup ----
    xp, wk8 = {}, {}
    for hl in ("hi", "lo"):
        xp[hl] = consts.tile([128, CP, 2, T], FP8, tag=f"x{hl}", name=f"x{hl}")
        wk8[hl] = consts.tile([128, CP, 2, cfg.NW], FP8, tag=f"w{hl}",
                              name=f"w{hl}")

    def x_dma_chunk(tch):
        # one batched DMA per hi/lo: [128, CP, 2, 512] column chunk of x^T
        cols = slice(tch * 512, (tch + 1) * 512)
        for hl in ("hi", "lo"):
            nc.sync.dma_start(
                xp[hl][:, :, :, cols],
                io[f"x8{hl}"].ap()[:, cols].rearrange(
                    "(cp k p) t -> p cp k t", cp=CP, k=2))

    nc.sync.dma_start(
        xp["hi"][:, :, :, 0:512],
        io["x8hi"].ap()[:, 0:512].rearrange("(cp k p) t -> p cp k t", cp=CP, k=2))
    nc.sync.dma_start(
        wk8["hi"][:],
        io["w8hi"].ap()[:].rearrange("(cp k p) n -> p cp k n", cp=CP, k=2))
    bk_t = consts.tile([128, NB], F32, tag="bk", name="bk")
    nc.sync.dma_start(bk_t[:], io["bk"].ap()[:].rearrange("(nb p) o -> p (nb o)",
                                                          nb=NB))
    tri_t = consts.tile([128, 128], FP16, tag="tri")
    nc.sync.dma_start(tri_t[:], io["tri"].ap())
    id_t = consts.tile([128, 128], FP16, tag="ident")
    nc.sync.dma_start(id_t[:], io["ident"].ap())
    nc.sync.dma_start(
        wk8["lo"][:],
        io["w8lo"].ap()[:].rearrange("(cp k p) n -> p cp k n", cp=CP, k=2))
    nc.sync.dma_start(
        xp["lo"][:, :, :, 0:512],
        io["x8lo"].ap()[:, 0:512].rearrange("(cp k p) t -> p cp k t", cp=CP, k=2))

    kt_t, kt8_t, y_t, v_t, wp_t = [], [], [], [], []
    for nb in range(NB):
        wp_t.append(consts.tile([128, COUT], FP16, tag=f"wp{nb}", name=f"wp{nb}"))
        kt_t.append(consts.tile([128, T], FP16, tag=f"kt{nb}", name=f"kt{nb}"))
        kt8_t.append(consts.tile([128, 2, T], FP8, tag=f"k8{nb}", name=f"k8{nb}"))
        y_t.append(consts.tile([128, T], FP16, tag=f"y{nb}", name=f"y{nb}"))
        # V slots per t-block: [ones(64) | K_e(64) | K_o(64) | ones(64)];
        # even-head lhsT = cols 0:128 -> rows [l_e; U_e],
        # odd-head  lhsT = cols 128:256 -> rows [U_o; l_o].
        v_t.append([consts.tile([128, 256], FP16, tag=f"v{nb}_{tb}",
                                name=f"v{nb}_{tb}") for tb in range(TB)])

    def wp_dma():
        # wp is only needed by proj (~40us in); keep it off the startup queue
        for nb in range(NB):
            nc.sync.dma_start(wp_t[nb][:],
                              io["wp"].ap()[nb * 128:(nb + 1) * 128, :])

    # warm the ACT exp table off the critical path
    warm = consts.tile([128, 1], F32, tag="warm", name="warm")
    nc.gpsimd.memset(warm[:], 0.0)
    nc.scalar.activation(warm[:], warm[:], EXP, scale=1.0)

    def kt_chunk(nb, tch):
        # KT[n, 512-chunk] = (x @ Wk*WS + bk*WS)^T via 12 DoubleRow matmuls
        # (hi@hi first so the lo-path DMAs can still be in flight), then
        # bias+cast (DVE) and fp8 hi/lo split (Pool).
        cols = slice(tch * 512, (tch + 1) * 512)
        ps = kpsum.tile([128, 512], F32, tag="kps", name="pskt")
        nsl = slice(nb * 128, (nb + 1) * 128)
        for i, (wv, xv) in enumerate((("hi", "hi"), ("lo", "hi"), ("hi", "lo"))):
            for cp in range(CP):
                nc.tensor.matmul(
                    ps[:], wk8[wv][:, cp, :, nsl], xp[xv][:, cp, :, cols],
                    start=(cp == 0 and i == 0), stop=(cp == CP - 1 and i == 2),
                    perf_mode=DR)
        nc.vector.tensor_scalar_add(kt_t[nb][:, cols], ps[:],
                                    bk_t[:, nb:nb + 1])
        nc.gpsimd.tensor_copy(kt8_t[nb][:, 0, cols], kt_t[nb][:, cols])
        nc.gpsimd.tensor_sub(kt8_t[nb][:, 1, cols], kt_t[nb][:, cols],
                             kt8_t[nb][:, 0, cols])

    def v_piece(nb, tb):
        # ones borders (Pool) + K middle from PE transpose, one DVE copy
        nc.gpsimd.memset(v_t[nb][tb][:, 0:64], 1.0)
        nc.gpsimd.memset(v_t[nb][tb][:, 192:256], 1.0)
        ps = kpsum.tile([128, 128], FP16, tag="kps", name="pst")
        nc.tensor.transpose(ps[:], kt_t[nb][:, tb * 128:(tb + 1) * 128], id_t[:])
        nc.vector.tensor_copy(v_t[nb][tb][:, 64:192], ps[:])

    SHIFT = list(range(32))  # identity mask: shift whole 64-partition window

    def strip_S(hp, ci, jb):
        # S^T strip [j-block jb, i-cols of chunk ci] for both heads of pair hp
        off = max(0, 128 * jb - 512 * ci)
        N = 512 - off
        ilo = 512 * ci + off
        ps = spsum.tile([128, 1024], F32, tag="sps", name="psS")
        for h, rows in ((0, slice(0, 64)), (1, slice(64, 128))):
            lhsT = kt8_t[hp][rows, :, jb * 128:(jb + 1) * 128]
            rhs = (kt8_t[hp][rows, 0, ilo:ilo + N]
                   .unsqueeze(1).broadcast_to([64, 2, N]))
            nc.tensor.matmul(ps[:, h * 512:h * 512 + N], lhsT, rhs,
                             start=True, stop=True, perf_mode=DR)
        et = epool.tile([128, 1024], FP16, tag="e", name="et")
        nc.scalar.activation(
            et[:].rearrange("p (a c) -> p a c", a=2)[:, :, 0:N],
            ps[:].rearrange("p (a c) -> p a c", a=2)[:, :, 0:N],
            EXP, scale=0.125 / (WS * WS))
        if jb >= 4 * ci:  # strip starts at the causal diagonal block
            nc.gpsimd.tensor_mul(et[:, 0:128], et[:, 0:128], tri_t[:])
            nc.gpsimd.tensor_mul(et[:, 512:640], et[:, 512:640], tri_t[:])
        return et, off, N

    def strip_PV(hp, jb, et, off, N, uA, uB, jmax):
        st, sp = (jb == 0), (jb == jmax - 1)
        nc.tensor.matmul(uA[:, off:off + N], v_t[hp][jb][:, 0:128],
                         et[:, 0:N], start=st, stop=sp)
        nc.tensor.matmul(uB[:, off:off + N], v_t[hp][jb][:, 128:256],
                         et[:, 512:512 + N], start=st, stop=sp)

    def normalize(hp, ci, uA, uB, drains):
        # uA rows: [l_e; U_e], uB rows: [U_o; l_o]; y rows: [U_o/l_o; U_e/l_e]
        # us-copies first (frees the u PSUM bank pair for the next unit),
        # then the pending proj drain, then recip/shuffle/scale on SBUF.
        cs = slice(ci * 512, (ci + 1) * 512)
        usA = upool.tile([128, 512], F32, tag="us", name="usA")
        nc.vector.tensor_copy(usA[:], uA[:])
        usB = upool.tile([128, 512], F32, tag="us", name="usB")
        nc.vector.tensor_copy(usB[:], uB[:])
        for d in drains:
            d()
        rA = rpool.tile([128, 512], F32, tag="r", name="rA")
        nc.vector.reciprocal(rA[0:64, :], usA[0:64, :])
        rB = rpool.tile([128, 512], F32, tag="r", name="rB")
        nc.vector.reciprocal(rB[64:128, :], usB[64:128, :])
        nc.vector.stream_shuffle(rA[64:128, :], rA[0:64, :], SHIFT)
        nc.vector.stream_shuffle(rB[0:64, :], rB[64:128, :], SHIFT)
        nc.vector.tensor_mul(y_t[hp][64:128, cs], usA[64:128, :], rA[64:128, :])
        nc.vector.tensor_mul(y_t[hp][0:64, cs], usB[0:64, :], rB[0:64, :])

    def proj_mm(tb, act_assist=False):
        # PE part of one proj row-block; returns a drain closure (copies +
        # out DMA).  act_assist splits the PSUM->SBUF copies across DVE and
        # ACT (tail only, when the exp stream is done).
        pos = []
        for nh in range(COUT // 512):
            po = kpsum.tile([128, 512], F32, tag="kps", name="po")
            for hp2 in range(NB):
                nc.tensor.matmul(po[:], y_t[hp2][:, tb * 128:(tb + 1) * 128],
                                 wp_t[hp2][:, nh * 512:(nh + 1) * 512],
                                 start=(hp2 == 0), stop=(hp2 == NB - 1))
            pos.append(po)

        def drain():
            ot = opool.tile([128, COUT], F32, tag="o", name="ot")
            for nh, po in enumerate(pos):
                if act_assist and nh == 1:
                    nc.scalar.copy(ot[:, nh * 512:(nh + 1) * 512], po[:])
                else:
                    nc.vector.tensor_copy(ot[:, nh * 512:(nh + 1) * 512], po[:])
            nc.sync.dma_start(io["out"].ap()[tb * 128:(tb + 1) * 128, :], ot[:])
        return drain

    # ---- global strip pipeline across all (ci, hp) units ----
    LA = 2
    units = [(ci, hp) for ci in range(NCH) for hp in range(NB)]
    kt_chunk(units[0][1], units[0][0])
    for tb in range(4):
        v_piece(units[0][1], tb)

    pend = []       # strips awaiting their PV: (hp, ci, jb, et, off, N, uA, uB, jmax)
    proj_pend = []  # row-blocks whose proj still needs emitting

    def pop_pv():
        hp0, ci0, jb0, et, off, N, uA0, uB0, jmax0 = pend.pop(0)
        strip_PV(hp0, jb0, et, off, N, uA0, uB0, jmax0)
        if jb0 == jmax0 - 1:  # unit finished: normalize (+ 1 proj drain)
            drains = []
            if proj_pend:
                drains.append(proj_mm(proj_pend.pop(0)))
            normalize(hp0, ci0, uA0, uB0, drains)
            if hp0 == NB - 1:
                proj_pend.extend(range(4 * ci0, 4 * ci0 + 4))

    for k, (ci, hp) in enumerate(units):
        jmax = (ci + 1) * 4
        uA = upsum.tile([128, 512], F32, tag="u", name="uA")
        uB = upsum.tile([128, 512], F32, tag="u", name="uB")
        for jb in range(jmax):
            et, off, N = strip_S(hp, ci, jb)
            pend.append((hp, ci, jb, et, off, N, uA, uB, jmax))
            if jb == 1 and k + 1 < len(units):
                nci, nhp = units[k + 1]
                if nhp == 0:
                    x_dma_chunk(nci)
                if k == 1:
                    wp_dma()
                kt_chunk(nhp, nci)
            if jb == 3 and k + 1 < len(units):
                nci, nhp = units[k + 1]
                for tb in range(4 * nci, 4 * nci + 4):
                    v_piece(nhp, tb)
            while len(pend) > LA:
                pop_pv()
    while pend:
        pop_pv()
    # tail: remaining proj row-blocks of the last chunk
    for tb in proj_pend:
        proj_mm(tb, act_assist=True)()


def make_inputs(cfg, x, Wk, bk, Wp):
    """Host-side input map for one core.
    x [T,CIN] fp32, Wk [CIN,NW], bk [NW], Wp [NW,COUT] (natural head order)."""
    import numpy as np
    import ml_dtypes
    E4 = ml_dtypes.float8_e4m3fn
    xT = np.ascontiguousarray(x.T).astype(np.float32)
    x8hi = xT.astype(E4)
    x8lo = (xT - x8hi.astype(np.float32)).astype(E4)
    Wks = (Wk * WS).astype(np.float32)
    w8hi = Wks.astype(E4)
    w8lo = (Wks - w8hi.astype(np.float32)).astype(E4)
    # wp rows per pair: [odd-head dims; even-head dims] to match y layout
    wp = (Wp / WS).astype(np.float32).reshape(cfg.NB, 2, 64, cfg.COUT)
    wp = np.ascontiguousarray(wp[:, ::-1].reshape(cfg.NW, cfg.COUT))
    jj, ii = np.meshgrid(np.arange(128), np.arange(128), indexing="ij")
    return {
        "x8hi": x8hi, "x8lo": x8lo, "w8hi": w8hi, "w8lo": w8lo,
        "bk": (bk * WS).reshape(-1, 1).astype(np.float32),
        "wp": wp.astype(np.float16),
        "tri": (jj <= ii).astype(np.float16),
        "ident": np.eye(128).astype(np.float16),
    }


# ======================================================================
# Host-side entry: shard across 8 NeuronCores as (batch x head-group),
# run the Bass kernel, gather + reduce partials on host.
# ======================================================================

import numpy as np

from concourse import bacc
from concourse.bass_utils import run_bass_kernel_spmd

B, T, C, H = 4, 2048, 1024, 16
N_CORES = 8
HG = 2                      # head groups (tensor-parallel axis)
NW = C // HG                # 512 columns of W_k per group

_cache = {}


def get_compiled():
    if "nc" not in _cache:
        cfg = Cfg(T=T, CIN=C, HL=H // HG, COUT=C)
        nc = bacc.Bacc("TRN2", target_bir_lowering=False, debug=False,
                       num_devices=N_CORES)
        io = declare_io(nc, cfg)
        with tile.TileContext(nc) as tc:
            with ExitStack() as ctx:
                build(ctx, tc, io, cfg)
        nc.compile()
        _cache["nc"] = (nc, cfg)
    return _cache["nc"]


def make_in_maps(cfg, x, W_attn, b_attn, W_proj):
    in_maps = []
    for core in range(N_CORES):
        b, hg = core // HG, core % HG
        sl = slice(C + hg * NW, C + (hg + 1) * NW)
        in_maps.append(make_inputs(
            cfg, x[b], W_attn[:, sl], b_attn[sl],
            W_proj[hg * NW:(hg + 1) * NW, :]))
    return in_maps


def kernel(x, W_attn, b_attn, W_proj, b_proj):
    x = np.asarray(x, dtype=np.float32)
    W_attn = np.asarray(W_attn, dtype=np.float32)
    b_attn = np.asarray(b_attn, dtype=np.float32)
    W_proj = np.asarray(W_proj, dtype=np.float32)
    b_proj = np.asarray(b_proj, dtype=np.float32)

    nc, cfg = get_compiled()
    in_maps = make_in_maps(cfg, x, W_attn, b_attn, W_proj)
    res = run_bass_kernel_spmd(nc, in_maps, core_ids=list(range(N_CORES)))
    out = np.empty((B, T, C), dtype=np.float32)
    for b in range(B):
        out[b] = res.results[HG * b]["out"] + res.results[HG * b + 1]["out"] \
            + b_proj[None, :]
    return out


# revision 17
# speedup vs baseline: 1.3244x; 1.0503x over previous
"""Per-core causal self-attention kernel (Bass/Tile, TRN2), v4.

One core's shard (batch b, head-group of HL=8 heads, reference quirk q=k=v):
    K  = x @ (32*Wk) + 32*bk              # [T, NW], NW = HL*64, scaled x32
    per head h: S = K_h K_h^T / (8*1024) (causal), P = softmax rows
    Y_h = P @ K_h                          # carries the x32 scale
    out_partial = Y @ (Wp/32)              # [T, COUT]; host sums partials

Speed structure:
  - K-gen in fp8 e4m3 hi/lo from host (x = x8hi+x8lo, W = w8hi+w8lo; W is
    pre-scaled x32 so the lo residuals stay out of fp8 subnormals), computed
    as 3 DoubleRow matmuls per 256-deep chunk pair: hi@hi + lo@hi + hi@lo.
  - S strips via ONE DoubleRow fp8 matmul per head: lhsT k-tiles are
    (kt8_hi, kt8_lo) of the j-block (exact to ~0.4%), rhs is kt8_hi of the
    i-columns broadcast over the k-tile dim (stride 0).  0.5 cycles/col.
  - PV + denominators in fp16: lhsT = V slot [ones|K_e] / [K_o|ones] from
    layout [ones|K_e|K_o|ones], built by ONE [128,128] DVE copy per
    (pair, t-block) from the PE transpose of kt.
  - tri-mask mults and fp16->fp8 kt splits on GpSimd (Pool); softmax
    normalize uses DVE stream_shuffle for the partition shift (no DMA).
  - GLOBAL software pipeline over all (chunk ci, pair hp, j-block) strips:
    S of strip g+2 is emitted before PV of strip g, ACROSS unit boundaries,
    so the in-order PE never makes the ACT exp stream wait.  kt chunks, V
    pieces and x DMAs are emitted just-in-time one unit ahead; proj of
    chunk ci's row-blocks rides the units of chunk ci+1.

Engine budget per core: ACT (exp) ~147us <- wall, PE ~140us, DVE ~105us,
Pool ~95us, DMA ~45us.
"""

from contextlib import ExitStack

import concourse.bass as bass
import concourse.tile as tile
from concourse import mybir

F32 = mybir.dt.float32
FP16 = mybir.dt.float16
FP8 = mybir.dt.float8e4
EXP = mybir.ActivationFunctionType.Exp
DR = mybir.MatmulPerfMode.DoubleRow

WS = 32.0  # host-side scale on Wk/bk (keeps fp8 lo-parts normal); /WS on Wp


class Cfg:
    def __init__(self, T=2048, CIN=1024, HL=8, COUT=1024):
        self.T, self.CIN, self.HL, self.COUT = T, CIN, HL, COUT
        assert HL % 2 == 0 and T % 512 == 0 and CIN % 256 == 0 and COUT % 512 == 0
        self.NW = HL * 64          # local head dims
        self.NB = self.NW // 128   # head-pair blocks (4)
        self.TB = T // 128         # t row-blocks (16)
        self.NCH = T // 512        # i chunks (4)
        self.CP = CIN // 256       # contraction chunk-pairs for DR K-gen (4)


def declare_io(nc, cfg):
    io = {}
    io["x8hi"] = nc.dram_tensor("x8hi", [cfg.CIN, cfg.T], FP8, kind="ExternalInput")
    io["x8lo"] = nc.dram_tensor("x8lo", [cfg.CIN, cfg.T], FP8, kind="ExternalInput")
    io["w8hi"] = nc.dram_tensor("w8hi", [cfg.CIN, cfg.NW], FP8, kind="ExternalInput")
    io["w8lo"] = nc.dram_tensor("w8lo", [cfg.CIN, cfg.NW], FP8, kind="ExternalInput")
    io["bk"] = nc.dram_tensor("bk", [cfg.NW, 1], F32, kind="ExternalInput")
    io["wp"] = nc.dram_tensor("wp", [cfg.NW, cfg.COUT], FP16, kind="ExternalInput")
    io["tri"] = nc.dram_tensor("tri", [128, 128], FP16, kind="ExternalInput")
    io["ident"] = nc.dram_tensor("ident", [128, 128], FP16, kind="ExternalInput")
    io["out"] = nc.dram_tensor("out", [cfg.T, cfg.COUT], FP16,
                               kind="ExternalOutput")
    return io


def build(ctx: ExitStack, tc: tile.TileContext, io, cfg: Cfg):
    nc = tc.nc
    T, HL, NB, TB, NCH, CP, COUT = (cfg.T, cfg.HL, cfg.NB, cfg.TB, cfg.NCH,
                                    cfg.CP, cfg.COUT)

    consts = ctx.enter_context(tc.tile_pool(name="consts", bufs=1))
    # PSUM (8 banks): s 2x[128,1024]f32=4, u 2x[128,512]f32=2, k 2x[128,512]=2
    spsum = ctx.enter_context(tc.tile_pool(name="sps", bufs=2, space="PSUM"))
    upsum = ctx.enter_context(tc.tile_pool(name="ups", bufs=2, space="PSUM"))
    kpsum = ctx.enter_context(tc.tile_pool(name="kps", bufs=2, space="PSUM"))
    upool = ctx.enter_context(tc.tile_pool(name="usb", bufs=4))
    epool = ctx.enter_context(tc.tile_pool(name="e", bufs=8))
    rpool = ctx.enter_context(tc.tile_pool(name="r", bufs=4))
    opool = ctx.enter_context(tc.tile_pool(name="o", bufs=3))

    # ---- persistent SBUF tensors; DMA order favors the hi-path startup ----
    xp, wk8 = {}, {}
    for hl in ("hi", "lo"):
        xp[hl] = consts.tile([128, CP, 2, T], FP8, tag=f"x{hl}", name=f"x{hl}")
        wk8[hl] = consts.tile([128, CP, 2, cfg.NW], FP8, tag=f"w{hl}",
                              name=f"w{hl}")

    def x_dma_chunk(tch):
        # one batched DMA per hi/lo: [128, CP, 2, 512] column chunk of x^T
        cols = slice(tch * 512, (tch + 1) * 512)
        for hl in ("hi", "lo"):
            nc.sync.dma_start(
                xp[hl][:, :, :, cols],
                io[f"x8{hl}"].ap()[:, cols].rearrange(
                    "(cp k p) t -> p cp k t", cp=CP, k=2))

    nc.sync.dma_start(
        xp["hi"][:, :, :, 0:512],
        io["x8hi"].ap()[:, 0:512].rearrange("(cp k p) t -> p cp k t", cp=CP, k=2))
    nc.sync.dma_start(
        wk8["hi"][:],
        io["w8hi"].ap()[:].rearrange("(cp k p) n -> p cp k n", cp=CP, k=2))
    bk_t = consts.tile([128, NB], F32, tag="bk", name="bk")
    nc.sync.dma_start(bk_t[:], io["bk"].ap()[:].rearrange("(nb p) o -> p (nb o)",
                                                          nb=NB))
    tri_t = consts.tile([128, 128], FP16, tag="tri")
    nc.sync.dma_start(tri_t[:], io["tri"].ap())
    id_t = consts.tile([128, 128], FP16, tag="ident")
    nc.sync.dma_start(id_t[:], io["ident"].ap())
    nc.sync.dma_start(
        wk8["lo"][:],
        io["w8lo"].ap()[:].rearrange("(cp k p) n -> p cp k n", cp=CP, k=2))
    nc.sync.dma_start(
        xp["lo"][:, :, :, 0:512],
        io["x8lo"].ap()[:, 0:512].rearrange("(cp k p) t -> p cp k t", cp=CP, k=2))

    kt_t, kt8_t, y_t, v_t, wp_t = [], [], [], [], []
    for nb in range(NB):
        wp_t.append(consts.tile([128, COUT], FP16, tag=f"wp{nb}", name=f"wp{nb}"))
        kt_t.append(consts.tile([128, T], FP16, tag=f"kt{nb}", name=f"kt{nb}"))
        kt8_t.append(consts.tile([128, 2, T], FP8, tag=f"k8{nb}", name=f"k8{nb}"))
        y_t.append(consts.tile([128, T], FP16, tag=f"y{nb}", name=f"y{nb}"))
        # V slots per t-block: [ones(64) | K_e(64) | K_o(64) | ones(64)];
        # even-head lhsT = cols 0:128 -> rows [l_e; U_e],
        # odd-head  lhsT = cols 128:256 -> rows [U_o; l_o].
        v_t.append([consts.tile([128, 256], FP16, tag=f"v{nb}_{tb}",
                                name=f"v{nb}_{tb}") for tb in range(TB)])

    def wp_dma():
        # wp is only needed by proj (~40us in); keep it off the startup queue
        for nb in range(NB):
            nc.sync.dma_start(wp_t[nb][:],
                              io["wp"].ap()[nb * 128:(nb + 1) * 128, :])

    # warm the ACT exp table off the critical path
    warm = consts.tile([128, 1], F32, tag="warm", name="warm")
    nc.gpsimd.memset(warm[:], 0.0)
    nc.scalar.activation(warm[:], warm[:], EXP, scale=1.0)

    def kt_chunk(nb, tch):
        # KT[n, 512-chunk] = (x @ Wk*WS + bk*WS)^T via 12 DoubleRow matmuls
        # (hi@hi first so the lo-path DMAs can still be in flight), then
        # bias+cast (DVE) and fp8 hi/lo split (Pool).
        cols = slice(tch * 512, (tch + 1) * 512)
        ps = kpsum.tile([128, 512], F32, tag="kps", name="pskt")
        nsl = slice(nb * 128, (nb + 1) * 128)
        for i, (wv, xv) in enumerate((("hi", "hi"), ("lo", "hi"), ("hi", "lo"))):
            for cp in range(CP):
                nc.tensor.matmul(
                    ps[:], wk8[wv][:, cp, :, nsl], xp[xv][:, cp, :, cols],
                    start=(cp == 0 and i == 0), stop=(cp == CP - 1 and i == 2),
                    perf_mode=DR)
        nc.vector.tensor_scalar_add(kt_t[nb][:, cols], ps[:],
                                    bk_t[:, nb:nb + 1])
        nc.gpsimd.tensor_copy(kt8_t[nb][:, 0, cols], kt_t[nb][:, cols])
        nc.gpsimd.tensor_sub(kt8_t[nb][:, 1, cols], kt_t[nb][:, cols],
                             kt8_t[nb][:, 0, cols])

    def v_piece(nb, tb):
        # ones borders via whole-tile DVE memset (K middle overwritten right
        # after by the one copy from the PE transpose); Pool stays free for
        # the latency-critical kt8 converts.
        nc.vector.memset(v_t[nb][tb][:], 1.0)
        ps = kpsum.tile([128, 128], FP16, tag="kps", name="pst")
        nc.tensor.transpose(ps[:], kt_t[nb][:, tb * 128:(tb + 1) * 128], id_t[:])
        nc.vector.tensor_copy(v_t[nb][tb][:, 64:192], ps[:])

    SHIFT = list(range(32))  # identity mask: shift whole 64-partition window

    def strip_S(hp, ci, jb):
        # S^T strip [j-block jb, i-cols of chunk ci] for both heads of pair hp
        off = max(0, 128 * jb - 512 * ci)
        N = 512 - off
        ilo = 512 * ci + off
        ps = spsum.tile([128, 1024], F32, tag="sps", name="psS")
        for h, rows in ((0, slice(0, 64)), (1, slice(64, 128))):
            lhsT = kt8_t[hp][rows, :, jb * 128:(jb + 1) * 128]
            rhs = (kt8_t[hp][rows, 0, ilo:ilo + N]
                   .unsqueeze(1).broadcast_to([64, 2, N]))
            nc.tensor.matmul(ps[:, h * 512:h * 512 + N], lhsT, rhs,
                             start=True, stop=True, perf_mode=DR)
        et = epool.tile([128, 1024], FP16, tag="e", name="et")
        nc.scalar.activation(
            et[:].rearrange("p (a c) -> p a c", a=2)[:, :, 0:N],
            ps[:].rearrange("p (a c) -> p a c", a=2)[:, :, 0:N],
            EXP, scale=0.125 / (WS * WS))
        if jb >= 4 * ci:  # strip starts at the causal diagonal block
            nc.gpsimd.tensor_mul(et[:, 0:128], et[:, 0:128], tri_t[:])
            nc.gpsimd.tensor_mul(et[:, 512:640], et[:, 512:640], tri_t[:])
        return et, off, N

    def strip_PV(hp, jb, et, off, N, uA, uB, jmax):
        st, sp = (jb == 0), (jb == jmax - 1)
        nc.tensor.matmul(uA[:, off:off + N], v_t[hp][jb][:, 0:128],
                         et[:, 0:N], start=st, stop=sp)
        nc.tensor.matmul(uB[:, off:off + N], v_t[hp][jb][:, 128:256],
                         et[:, 512:512 + N], start=st, stop=sp)

    def normalize(hp, ci, uA, uB, drains):
        # uA rows: [l_e; U_e], uB rows: [U_o; l_o]; y rows: [U_o/l_o; U_e/l_e]
        # us-copies first (frees the u PSUM bank pair for the next unit),
        # then the pending proj drain, then recip/shuffle/scale on SBUF.
        cs = slice(ci * 512, (ci + 1) * 512)
        usA = upool.tile([128, 512], F32, tag="us", name="usA")
        nc.vector.tensor_copy(usA[:], uA[:])
        usB = upool.tile([128, 512], F32, tag="us", name="usB")
        nc.vector.tensor_copy(usB[:], uB[:])
        for d in drains:
            d()
        rA = rpool.tile([128, 512], F32, tag="r", name="rA")
        nc.vector.reciprocal(rA[0:64, :], usA[0:64, :])
        rB = rpool.tile([128, 512], F32, tag="r", name="rB")
        nc.vector.reciprocal(rB[64:128, :], usB[64:128, :])
        nc.vector.stream_shuffle(rA[64:128, :], rA[0:64, :], SHIFT)
        nc.vector.stream_shuffle(rB[0:64, :], rB[64:128, :], SHIFT)
        nc.vector.tensor_mul(y_t[hp][64:128, cs], usA[64:128, :], rA[64:128, :])
        nc.vector.tensor_mul(y_t[hp][0:64, cs], usB[0:64, :], rB[0:64, :])

    def proj_mm(tb, act_assist=False):
        # PE part of one proj row-block; returns a drain closure (copies +
        # out DMA).  act_assist splits the PSUM->SBUF copies across DVE and
        # ACT (tail only, when the exp stream is done).
        pos = []
        for nh in range(COUT // 512):
            po = kpsum.tile([128, 512], F32, tag="kps", name="po")
            for hp2 in range(NB):
                nc.tensor.matmul(po[:], y_t[hp2][:, tb * 128:(tb + 1) * 128],
                                 wp_t[hp2][:, nh * 512:(nh + 1) * 512],
                                 start=(hp2 == 0), stop=(hp2 == NB - 1))
            pos.append(po)

        def drain():
            ot = opool.tile([128, COUT], FP16, tag="o", name="ot")
            for nh, po in enumerate(pos):
                if act_assist and nh == 1:
                    nc.scalar.copy(ot[:, nh * 512:(nh + 1) * 512], po[:])
                else:
                    nc.vector.tensor_copy(ot[:, nh * 512:(nh + 1) * 512], po[:])
            nc.sync.dma_start(io["out"].ap()[tb * 128:(tb + 1) * 128, :], ot[:])
        return drain

    # ---- global strip pipeline across all (ci, hp) units ----
    LA = 2
    units = [(ci, hp) for ci in range(NCH) for hp in range(NB)]

    def prep_unit(k):
        # kt chunk for units[k], two units ahead of use; x column DMAs for
        # the next chunk go out one prep earlier still so the kt matmuls
        # never sit on the PE queue waiting for a transfer.
        nci, nhp = units[k]
        if nhp == NB - 1 and nci + 1 < NCH:
            x_dma_chunk(nci + 1)
        kt_chunk(nhp, nci)

    prep_unit(0)
    prep_unit(1)
    for tb in range(4):
        v_piece(units[0][1], tb)

    pend = []       # strips awaiting their PV: (hp, ci, jb, et, off, N, uA, uB, jmax)
    proj_pend = []  # row-blocks whose proj still needs emitting

    def pop_pv():
        hp0, ci0, jb0, et, off, N, uA0, uB0, jmax0 = pend.pop(0)
        strip_PV(hp0, jb0, et, off, N, uA0, uB0, jmax0)
        if jb0 == jmax0 - 1:  # unit finished: normalize (+ 1 proj drain)
            drains = []
            if proj_pend:
                drains.append(proj_mm(proj_pend.pop(0)))
            normalize(hp0, ci0, uA0, uB0, drains)
            if hp0 == NB - 1:
                proj_pend.extend(range(4 * ci0, 4 * ci0 + 4))

    for k, (ci, hp) in enumerate(units):
        jmax = (ci + 1) * 4
        uA = upsum.tile([128, 512], F32, tag="u", name="uA")
        uB = upsum.tile([128, 512], F32, tag="u", name="uB")
        for jb in range(jmax):
            et, off, N = strip_S(hp, ci, jb)
            pend.append((hp, ci, jb, et, off, N, uA, uB, jmax))
            if jb == 1:
                if k == 1:
                    wp_dma()
                if k + 2 < len(units):
                    prep_unit(k + 2)
            if jb == 3 and k + 1 < len(units):
                nci, nhp = units[k + 1]
                for tb in range(4 * nci, 4 * nci + 4):
                    v_piece(nhp, tb)
            while len(pend) > LA:
                pop_pv()
    while pend:
        pop_pv()
    # tail: remaining proj row-blocks of the last chunk
    for tb in proj_pend:
        proj_mm(tb, act_assist=True)()


def make_inputs(cfg, x, Wk, bk, Wp):
    """Host-side input map for one core.
    x [T,CIN] fp32, Wk [CIN,NW], bk [NW], Wp [NW,COUT] (natural head order)."""
    import numpy as np
    import ml_dtypes
    E4 = ml_dtypes.float8_e4m3fn
    xT = np.ascontiguousarray(x.T).astype(np.float32)
    x8hi = xT.astype(E4)
    x8lo = (xT - x8hi.astype(np.float32)).astype(E4)
    Wks = (Wk * WS).astype(np.float32)
    w8hi = Wks.astype(E4)
    w8lo = (Wks - w8hi.astype(np.float32)).astype(E4)
    # wp rows per pair: [odd-head dims; even-head dims] to match y layout
    wp = (Wp / WS).astype(np.float32).reshape(cfg.NB, 2, 64, cfg.COUT)
    wp = np.ascontiguousarray(wp[:, ::-1].reshape(cfg.NW, cfg.COUT))
    jj, ii = np.meshgrid(np.arange(128), np.arange(128), indexing="ij")
    return {
        "x8hi": x8hi, "x8lo": x8lo, "w8hi": w8hi, "w8lo": w8lo,
        "bk": (bk * WS).reshape(-1, 1).astype(np.float32),
        "wp": wp.astype(np.float16),
        "tri": (jj <= ii).astype(np.float16),
        "ident": np.eye(128).astype(np.float16),
    }


# ======================================================================
# Host-side entry: shard across 8 NeuronCores as (batch x head-group),
# run the Bass kernel, gather + reduce partials on host.
# ======================================================================

import numpy as np

from concourse import bacc
from concourse.bass_utils import run_bass_kernel_spmd

B, T, C, H = 4, 2048, 1024, 16
N_CORES = 8
HG = 2                      # head groups (tensor-parallel axis)
NW = C // HG                # 512 columns of W_k per group

_cache = {}


def get_compiled():
    if "nc" not in _cache:
        cfg = Cfg(T=T, CIN=C, HL=H // HG, COUT=C)
        nc = bacc.Bacc("TRN2", target_bir_lowering=False, debug=False,
                       num_devices=N_CORES)
        io = declare_io(nc, cfg)
        with tile.TileContext(nc) as tc:
            with ExitStack() as ctx:
                build(ctx, tc, io, cfg)
        nc.compile()
        _cache["nc"] = (nc, cfg)
    return _cache["nc"]


def make_in_maps(cfg, x, W_attn, b_attn, W_proj):
    in_maps = []
    for core in range(N_CORES):
        b, hg = core // HG, core % HG
        sl = slice(C + hg * NW, C + (hg + 1) * NW)
        in_maps.append(make_inputs(
            cfg, x[b], W_attn[:, sl], b_attn[sl],
            W_proj[hg * NW:(hg + 1) * NW, :]))
    return in_maps


def kernel(x, W_attn, b_attn, W_proj, b_proj):
    x = np.asarray(x, dtype=np.float32)
    W_attn = np.asarray(W_attn, dtype=np.float32)
    b_attn = np.asarray(b_attn, dtype=np.float32)
    W_proj = np.asarray(W_proj, dtype=np.float32)
    b_proj = np.asarray(b_proj, dtype=np.float32)

    nc, cfg = get_compiled()
    in_maps = make_in_maps(cfg, x, W_attn, b_attn, W_proj)
    res = run_bass_kernel_spmd(nc, in_maps, core_ids=list(range(N_CORES)))
    out = np.empty((B, T, C), dtype=np.float32)
    for b in range(B):
        out[b] = res.results[HG * b]["out"].astype(np.float32) \
            + res.results[HG * b + 1]["out"].astype(np.float32) \
            + b_proj[None, :]
    return out


# revision 22
# speedup vs baseline: 1.3323x; 1.0060x over previous
"""Per-core causal self-attention kernel (Bass/Tile, TRN2), v4.

One core's shard (batch b, head-group of HL=8 heads, reference quirk q=k=v):
    K  = x @ (32*Wk) + 32*bk              # [T, NW], NW = HL*64, scaled x32
    per head h: S = K_h K_h^T / (8*1024) (causal), P = softmax rows
    Y_h = P @ K_h                          # carries the x32 scale
    out_partial = Y @ (Wp/32)              # [T, COUT]; host sums partials

Speed structure:
  - K-gen in fp8 e4m3 hi/lo from host (x = x8hi+x8lo, W = w8hi+w8lo; W is
    pre-scaled x32 so the lo residuals stay out of fp8 subnormals), computed
    as 3 DoubleRow matmuls per 256-deep chunk pair: hi@hi + lo@hi + hi@lo.
  - S strips via ONE DoubleRow fp8 matmul per head: lhsT k-tiles are
    (kt8_hi, kt8_lo) of the j-block (exact to ~0.4%), rhs is kt8_hi of the
    i-columns broadcast over the k-tile dim (stride 0).  0.5 cycles/col.
  - PV + denominators in fp16: lhsT = V slot [ones|K_e] / [K_o|ones] from
    layout [ones|K_e|K_o|ones], built by ONE [128,128] DVE copy per
    (pair, t-block) from the PE transpose of kt.
  - tri-mask mults and fp16->fp8 kt splits on GpSimd (Pool); softmax
    normalize uses DVE stream_shuffle for the partition shift (no DMA).
  - GLOBAL software pipeline over all (chunk ci, pair hp, j-block) strips:
    S of strip g+2 is emitted before PV of strip g, ACROSS unit boundaries,
    so the in-order PE never makes the ACT exp stream wait.  kt chunks, V
    pieces and x DMAs are emitted just-in-time one unit ahead; proj of
    chunk ci's row-blocks rides the units of chunk ci+1.

Engine budget per core: ACT (exp) ~147us <- wall, PE ~140us, DVE ~105us,
Pool ~95us, DMA ~45us.
"""

from contextlib import ExitStack

import concourse.bass as bass
import concourse.tile as tile
from concourse import mybir

F32 = mybir.dt.float32
FP16 = mybir.dt.float16
FP8 = mybir.dt.float8e4
EXP = mybir.ActivationFunctionType.Exp
DR = mybir.MatmulPerfMode.DoubleRow

WS = 32.0  # host-side scale on Wk/bk (keeps fp8 lo-parts normal); /WS on Wp


class Cfg:
    def __init__(self, T=2048, CIN=1024, HL=8, COUT=1024):
        self.T, self.CIN, self.HL, self.COUT = T, CIN, HL, COUT
        assert HL % 2 == 0 and T % 512 == 0 and CIN % 256 == 0 and COUT % 512 == 0
        self.NW = HL * 64          # local head dims
        self.NB = self.NW // 128   # head-pair blocks (4)
        self.TB = T // 128         # t row-blocks (16)
        self.NCH = T // 512        # i chunks (4)
        self.CP = CIN // 256       # contraction chunk-pairs for DR K-gen (4)


def declare_io(nc, cfg):
    io = {}
    io["x8hi"] = nc.dram_tensor("x8hi", [cfg.CIN, cfg.T], FP8, kind="ExternalInput")
    io["x8lo"] = nc.dram_tensor("x8lo", [cfg.CIN, cfg.T], FP8, kind="ExternalInput")
    io["w8hi"] = nc.dram_tensor("w8hi", [cfg.CIN, cfg.NW], FP8, kind="ExternalInput")
    io["w8lo"] = nc.dram_tensor("w8lo", [cfg.CIN, cfg.NW], FP8, kind="ExternalInput")
    io["bk"] = nc.dram_tensor("bk", [cfg.NW, 1], F32, kind="ExternalInput")
    io["wp"] = nc.dram_tensor("wp", [cfg.NW, cfg.COUT], FP16, kind="ExternalInput")
    io["tri"] = nc.dram_tensor("tri", [128, 128], FP16, kind="ExternalInput")
    io["ident"] = nc.dram_tensor("ident", [128, 128], FP16, kind="ExternalInput")
    io["out"] = nc.dram_tensor("out", [cfg.T, cfg.COUT], FP16,
                               kind="ExternalOutput")
    return io


def build(ctx: ExitStack, tc: tile.TileContext, io, cfg: Cfg):
    nc = tc.nc
    T, HL, NB, TB, NCH, CP, COUT = (cfg.T, cfg.HL, cfg.NB, cfg.TB, cfg.NCH,
                                    cfg.CP, cfg.COUT)

    consts = ctx.enter_context(tc.tile_pool(name="consts", bufs=1))
    # PSUM (8 banks): s 2x[128,1024]f32=4, u 2x[128,512]f32=2, k 2x[128,512]=2
    spsum = ctx.enter_context(tc.tile_pool(name="sps", bufs=2, space="PSUM"))
    upsum = ctx.enter_context(tc.tile_pool(name="ups", bufs=2, space="PSUM"))
    kpsum = ctx.enter_context(tc.tile_pool(name="kps", bufs=2, space="PSUM"))
    upool = ctx.enter_context(tc.tile_pool(name="usb", bufs=4))
    epool = ctx.enter_context(tc.tile_pool(name="e", bufs=8))
    rpool = ctx.enter_context(tc.tile_pool(name="r", bufs=4))
    opool = ctx.enter_context(tc.tile_pool(name="o", bufs=3))

    # ---- persistent SBUF tensors; DMA order favors the hi-path startup ----
    xp, wk8 = {}, {}
    for hl in ("hi", "lo"):
        xp[hl] = consts.tile([128, CP, 2, T], FP8, tag=f"x{hl}", name=f"x{hl}")
        wk8[hl] = consts.tile([128, CP, 2, cfg.NW], FP8, tag=f"w{hl}",
                              name=f"w{hl}")

    def x_dma_chunk(tch):
        # one batched DMA per hi/lo: [128, CP, 2, 512] column chunk of x^T
        cols = slice(tch * 512, (tch + 1) * 512)
        for hl in ("hi", "lo"):
            nc.sync.dma_start(
                xp[hl][:, :, :, cols],
                io[f"x8{hl}"].ap()[:, cols].rearrange(
                    "(cp k p) t -> p cp k t", cp=CP, k=2))

    nc.sync.dma_start(
        xp["hi"][:, :, :, 0:512],
        io["x8hi"].ap()[:, 0:512].rearrange("(cp k p) t -> p cp k t", cp=CP, k=2))
    nc.sync.dma_start(
        wk8["hi"][:],
        io["w8hi"].ap()[:].rearrange("(cp k p) n -> p cp k n", cp=CP, k=2))

    # PE warm-up: zero matmuls ramp the tensor engine to full p-state while
    # the first x/w DMAs land, so the opening kt chunk runs at 2.4 GHz.
    zt = consts.tile([128, 640], FP8, tag="zt", name="zt")
    nc.vector.memset(zt[:], 0.0)

    def pe_warm(n):
        for _ in range(n):
            ps = kpsum.tile([128, 512], F32, tag="kps", name="warmmm")
            nc.tensor.matmul(ps[:], zt[0:64, 0:128], zt[0:64, 128:640],
                             start=True, stop=True)

    pe_warm(12)
    bk_t = consts.tile([128, NB], F32, tag="bk", name="bk")
    nc.sync.dma_start(bk_t[:], io["bk"].ap()[:].rearrange("(nb p) o -> p (nb o)",
                                                          nb=NB))
    tri_t = consts.tile([128, 128], FP16, tag="tri")
    nc.sync.dma_start(tri_t[:], io["tri"].ap())
    id_t = consts.tile([128, 128], FP16, tag="ident")
    nc.sync.dma_start(id_t[:], io["ident"].ap())
    nc.sync.dma_start(
        wk8["lo"][:],
        io["w8lo"].ap()[:].rearrange("(cp k p) n -> p cp k n", cp=CP, k=2))
    nc.sync.dma_start(
        xp["lo"][:, :, :, 0:512],
        io["x8lo"].ap()[:, 0:512].rearrange("(cp k p) t -> p cp k t", cp=CP, k=2))

    kt_t, kt8_t, y_t, v_t, wp_t = [], [], [], [], []
    for nb in range(NB):
        wp_t.append(consts.tile([128, COUT], FP16, tag=f"wp{nb}", name=f"wp{nb}"))
        kt_t.append(consts.tile([128, T], FP16, tag=f"kt{nb}", name=f"kt{nb}"))
        kt8_t.append(consts.tile([128, 2, T], FP8, tag=f"k8{nb}", name=f"k8{nb}"))
        y_t.append(consts.tile([128, T], FP16, tag=f"y{nb}", name=f"y{nb}"))
        # V slots per t-block: [ones(64) | K_e(64) | K_o(64) | ones(64)];
        # even-head lhsT = cols 0:128 -> rows [l_e; U_e],
        # odd-head  lhsT = cols 128:256 -> rows [U_o; l_o].
        v_t.append([consts.tile([128, 256], FP16, tag=f"v{nb}_{tb}",
                                name=f"v{nb}_{tb}") for tb in range(TB)])

    def wp_dma():
        # wp is only needed by proj (~40us in); keep it off the startup queue
        for nb in range(NB):
            nc.sync.dma_start(wp_t[nb][:],
                              io["wp"].ap()[nb * 128:(nb + 1) * 128, :])

    # warm the ACT exp table off the critical path
    warm = consts.tile([128, 1], F32, tag="warm", name="warm")
    nc.gpsimd.memset(warm[:], 0.0)
    nc.scalar.activation(warm[:], warm[:], EXP, scale=1.0)

    def kt_chunk(nb, tch):
        # KT[n, 512-chunk] = (x @ Wk*WS + bk*WS)^T via 12 DoubleRow matmuls
        # (hi@hi first so the lo-path DMAs can still be in flight), then
        # bias+cast (DVE) and fp8 hi/lo split (Pool).
        cols = slice(tch * 512, (tch + 1) * 512)
        ps = kpsum.tile([128, 512], F32, tag="kps", name="pskt")
        nsl = slice(nb * 128, (nb + 1) * 128)
        for i, (wv, xv) in enumerate((("hi", "hi"), ("lo", "hi"), ("hi", "lo"))):
            for cp in range(CP):
                nc.tensor.matmul(
                    ps[:], wk8[wv][:, cp, :, nsl], xp[xv][:, cp, :, cols],
                    start=(cp == 0 and i == 0), stop=(cp == CP - 1 and i == 2),
                    perf_mode=DR)
        nc.vector.tensor_scalar_add(kt_t[nb][:, cols], ps[:],
                                    bk_t[:, nb:nb + 1])
        nc.gpsimd.tensor_copy(kt8_t[nb][:, 0, cols], kt_t[nb][:, cols])
        nc.gpsimd.tensor_sub(kt8_t[nb][:, 1, cols], kt_t[nb][:, cols],
                             kt8_t[nb][:, 0, cols])

    def v_piece(nb, tb):
        # ones borders via whole-tile DVE memset (K middle overwritten right
        # after by the one copy from the PE transpose); Pool stays free for
        # the latency-critical kt8 converts.
        nc.vector.memset(v_t[nb][tb][:], 1.0)
        ps = kpsum.tile([128, 128], FP16, tag="kps", name="pst")
        nc.tensor.transpose(ps[:], kt_t[nb][:, tb * 128:(tb + 1) * 128], id_t[:])
        nc.vector.tensor_copy(v_t[nb][tb][:, 64:192], ps[:])

    SHIFT = list(range(32))  # identity mask: shift whole 64-partition window

    def strip_S(hp, ci, jb):
        # S^T strip [j-block jb, i-cols of chunk ci] for both heads of pair hp
        off = max(0, 128 * jb - 512 * ci)
        N = 512 - off
        ilo = 512 * ci + off
        ps = spsum.tile([128, 1024], F32, tag="sps", name="psS")
        for h, rows in ((0, slice(0, 64)), (1, slice(64, 128))):
            lhsT = kt8_t[hp][rows, :, jb * 128:(jb + 1) * 128]
            rhs = (kt8_t[hp][rows, 0, ilo:ilo + N]
                   .unsqueeze(1).broadcast_to([64, 2, N]))
            nc.tensor.matmul(ps[:, h * 512:h * 512 + N], lhsT, rhs,
                             start=True, stop=True, perf_mode=DR)
        et = epool.tile([128, 1024], FP16, tag="e", name="et")
        nc.scalar.activation(
            et[:].rearrange("p (a c) -> p a c", a=2)[:, :, 0:N],
            ps[:].rearrange("p (a c) -> p a c", a=2)[:, :, 0:N],
            EXP, scale=0.125 / (WS * WS))
        if jb >= 4 * ci:  # strip starts at the causal diagonal block
            nc.gpsimd.tensor_mul(et[:, 0:128], et[:, 0:128], tri_t[:])
            nc.gpsimd.tensor_mul(et[:, 512:640], et[:, 512:640], tri_t[:])
        return et, off, N

    def strip_PV(hp, jb, et, off, N, uA, uB, jmax):
        st, sp = (jb == 0), (jb == jmax - 1)
        nc.tensor.matmul(uA[:, off:off + N], v_t[hp][jb][:, 0:128],
                         et[:, 0:N], start=st, stop=sp)
        nc.tensor.matmul(uB[:, off:off + N], v_t[hp][jb][:, 128:256],
                         et[:, 512:512 + N], start=st, stop=sp)

    def normalize(hp, ci, uA, uB, drains, final=False):
        # uA rows: [l_e; U_e], uB rows: [U_o; l_o]; y rows: [U_o/l_o; U_e/l_e]
        # us-copies first (frees the u PSUM bank pair for the next unit),
        # then the pending proj drain, then recip/shuffle/scale on SBUF.
        # The final unit skips the copies (nobody needs its banks again).
        cs = slice(ci * 512, (ci + 1) * 512)
        if final:
            usA, usB = uA, uB
        else:
            usA = upool.tile([128, 512], F32, tag="us", name="usA")
            nc.vector.tensor_copy(usA[:], uA[:])
            usB = upool.tile([128, 512], F32, tag="us", name="usB")
            nc.vector.tensor_copy(usB[:], uB[:])
        for d in drains:
            d()
        rA = rpool.tile([128, 512], F32, tag="r", name="rA")
        nc.vector.reciprocal(rA[0:64, :], usA[0:64, :])
        rB = rpool.tile([128, 512], F32, tag="r", name="rB")
        nc.vector.reciprocal(rB[64:128, :], usB[64:128, :])
        nc.vector.stream_shuffle(rA[64:128, :], rA[0:64, :], SHIFT)
        nc.vector.stream_shuffle(rB[0:64, :], rB[64:128, :], SHIFT)
        nc.vector.tensor_mul(y_t[hp][64:128, cs], usA[64:128, :], rA[64:128, :])
        nc.vector.tensor_mul(y_t[hp][0:64, cs], usB[0:64, :], rB[0:64, :])

    def proj_mm(tb, act_assist=False):
        # PE part of one proj row-block; returns a drain closure (copies +
        # out DMA).  act_assist splits the PSUM->SBUF copies across DVE and
        # ACT (tail only, when the exp stream is done).
        pos = []
        for nh in range(COUT // 512):
            po = kpsum.tile([128, 512], F32, tag="kps", name="po")
            for hp2 in range(NB):
                nc.tensor.matmul(po[:], y_t[hp2][:, tb * 128:(tb + 1) * 128],
                                 wp_t[hp2][:, nh * 512:(nh + 1) * 512],
                                 start=(hp2 == 0), stop=(hp2 == NB - 1))
            pos.append(po)

        def drain():
            ot = opool.tile([128, COUT], FP16, tag="o", name="ot")
            for nh, po in enumerate(pos):
                if act_assist and nh == 1:
                    nc.scalar.copy(ot[:, nh * 512:(nh + 1) * 512], po[:])
                else:
                    nc.vector.tensor_copy(ot[:, nh * 512:(nh + 1) * 512], po[:])
            nc.sync.dma_start(io["out"].ap()[tb * 128:(tb + 1) * 128, :], ot[:])
        return drain

    # ---- global strip pipeline across all (ci, hp) units ----
    LA = 2
    units = [(ci, hp) for ci in range(NCH) for hp in range(NB)]

    def prep_unit(k):
        # kt chunk for units[k], two units ahead of use; x column DMAs for
        # the next chunk go out one prep earlier still so the kt matmuls
        # never sit on the PE queue waiting for a transfer.
        nci, nhp = units[k]
        if nhp == NB - 1 and nci + 1 < NCH:
            x_dma_chunk(nci + 1)
        kt_chunk(nhp, nci)

    PREP_AHEAD = 3
    for j in range(PREP_AHEAD):
        prep_unit(j)
    for tb in range(4):
        v_piece(units[0][1], tb)

    pend = []       # strips awaiting their PV: (hp, ci, jb, et, off, N, uA, uB, jmax)
    proj_pend = []  # row-blocks whose proj still needs emitting

    def pop_pv():
        hp0, ci0, jb0, et, off, N, uA0, uB0, jmax0 = pend.pop(0)
        strip_PV(hp0, jb0, et, off, N, uA0, uB0, jmax0)
        if jb0 == jmax0 - 1:  # unit finished: normalize (+ 1 proj drain)
            final = not pend and ci0 == NCH - 1 and hp0 == NB - 1
            if final:
                pe_warm(10)  # keep the PE p-state up through the tail proj
            drains = []
            if proj_pend:
                drains.append(proj_mm(proj_pend.pop(0)))
            normalize(hp0, ci0, uA0, uB0, drains, final=final)
            if hp0 == NB - 1:
                proj_pend.extend(range(4 * ci0, 4 * ci0 + 4))

    for k, (ci, hp) in enumerate(units):
        jmax = (ci + 1) * 4
        uA = upsum.tile([128, 512], F32, tag="u", name="uA")
        uB = upsum.tile([128, 512], F32, tag="u", name="uB")
        for jb in range(jmax):
            et, off, N = strip_S(hp, ci, jb)
            pend.append((hp, ci, jb, et, off, N, uA, uB, jmax))
            if jb == 1:
                if k == 1:
                    wp_dma()
                if k + PREP_AHEAD < len(units):
                    prep_unit(k + PREP_AHEAD)
            if jb == 3 and k + 1 < len(units):
                nci, nhp = units[k + 1]
                for tb in range(4 * nci, 4 * nci + 4):
                    v_piece(nhp, tb)
            while len(pend) > LA:
                pop_pv()
    while pend:
        pop_pv()
    # tail: remaining proj row-blocks of the last chunk
    for tb in proj_pend:
        proj_mm(tb, act_assist=True)()


def make_inputs(cfg, x, Wk, bk, Wp):
    """Host-side input map for one core.
    x [T,CIN] fp32, Wk [CIN,NW], bk [NW], Wp [NW,COUT] (natural head order)."""
    import numpy as np
    import ml_dtypes
    E4 = ml_dtypes.float8_e4m3fn
    xT = np.ascontiguousarray(x.T).astype(np.float32)
    x8hi = xT.astype(E4)
    x8lo = (xT - x8hi.astype(np.float32)).astype(E4)
    Wks = (Wk * WS).astype(np.float32)
    w8hi = Wks.astype(E4)
    w8lo = (Wks - w8hi.astype(np.float32)).astype(E4)
    # wp rows per pair: [odd-head dims; even-head dims] to match y layout
    wp = (Wp / WS).astype(np.float32).reshape(cfg.NB, 2, 64, cfg.COUT)
    wp = np.ascontiguousarray(wp[:, ::-1].reshape(cfg.NW, cfg.COUT))
    jj, ii = np.meshgrid(np.arange(128), np.arange(128), indexing="ij")
    return {
        "x8hi": x8hi, "x8lo": x8lo, "w8hi": w8hi, "w8lo": w8lo,
        "bk": (bk * WS).reshape(-1, 1).astype(np.float32),
        "wp": wp.astype(np.float16),
        "tri": (jj <= ii).astype(np.float16),
        "ident": np.eye(128).astype(np.float16),
    }


# ======================================================================
# Host-side entry: shard across 8 NeuronCores as (batch x head-group),
# run the Bass kernel, gather + reduce partials on host.
# ======================================================================

import numpy as np

from concourse import bacc
from concourse.bass_utils import run_bass_kernel_spmd

B, T, C, H = 4, 2048, 1024, 16
N_CORES = 8
HG = 2                      # head groups (tensor-parallel axis)
NW = C // HG                # 512 columns of W_k per group

_cache = {}


def get_compiled():
    if "nc" not in _cache:
        cfg = Cfg(T=T, CIN=C, HL=H // HG, COUT=C)
        nc = bacc.Bacc("TRN2", target_bir_lowering=False, debug=False,
                       num_devices=N_CORES)
        io = declare_io(nc, cfg)
        with tile.TileContext(nc) as tc:
            with ExitStack() as ctx:
                build(ctx, tc, io, cfg)
        nc.compile()
        _cache["nc"] = (nc, cfg)
    return _cache["nc"]


def make_in_maps(cfg, x, W_attn, b_attn, W_proj):
    in_maps = []
    for core in range(N_CORES):
        b, hg = core // HG, core % HG
        sl = slice(C + hg * NW, C + (hg + 1) * NW)
        in_maps.append(make_inputs(
            cfg, x[b], W_attn[:, sl], b_attn[sl],
            W_proj[hg * NW:(hg + 1) * NW, :]))
    return in_maps


def kernel(x, W_attn, b_attn, W_proj, b_proj):
    x = np.asarray(x, dtype=np.float32)
    W_attn = np.asarray(W_attn, dtype=np.float32)
    b_attn = np.asarray(b_attn, dtype=np.float32)
    W_proj = np.asarray(W_proj, dtype=np.float32)
    b_proj = np.asarray(b_proj, dtype=np.float32)

    nc, cfg = get_compiled()
    in_maps = make_in_maps(cfg, x, W_attn, b_attn, W_proj)
    res = run_bass_kernel_spmd(nc, in_maps, core_ids=list(range(N_CORES)))
    out = np.empty((B, T, C), dtype=np.float32)
    for b in range(B):
        out[b] = res.results[HG * b]["out"].astype(np.float32) \
            + res.results[HG * b + 1]["out"].astype(np.float32) \
            + b_proj[None, :]
    return out
